# revision 49
# baseline (speedup 1.0000x reference)
"""Causal attention head on 8 trn2 NeuronCores.

Sharding: core c = (batch b = c//2, type t = c%2). Each core handles 4
query stripes of 512 of its batch. Causal balance: type A gets stripes
[7,5,2,0] with real key-block counts R_A=[32,24,12,4]; type B stripes
[6,4,3,1] with R_B=[28,20,16,8]. One SPMD program: every core runs the
padded template T=[32,24,16,8]; per-core behaviour comes only from input
data (per-core threshold scalars select ones/triangle/zero mask tiles).

Everything on the PE array is bf16 (fp32 HIGH poisons fast-weight-load
and fp32 moving operands stream at half rate). Score matmuls pack two
key-blocks per issue via PE row tiling: kT pairs live on partition
halves 0:64 / 64:128 (host interleaves ek into even/odd block regions),
qT is duplicated onto both halves by a column-duplicated Wq.

The program is a 4-phase pipeline, one phase per slot (processed small
to large: j=3,2,1,0). Phase p: DMA group for phase p+2, projections for
just the new kTp chunk / qT chunk / a quota of v blocks this slot needs,
4 mask tiles mk[t]=(QK>=th[t]) (QK is a gpsimd iota qi-kp-128*i2; th in
{-1e9,0,256,1e9} selects ones/triangle/zero), then the slot's attention.
Input DMAs are split across the SP and ACT issue queues; a handful of
dummy matmuls at the start keep the PE busy under the DMA wait so the
HAM clock-gate opens (1.2->2.4 GHz) before real work.

Per pair d (= template positions 2d, 2d+1) of slot j:
  ps[:,   0: 512] = kTp[ 0: 64, d].T @ qT[ 0: 64, slot]   (rows 0-63)
  ps[:, 512:1024] = kTp[64:128, d].T @ qT[64:128, slot]   (rows 64-127)
  e = exp(0.125*ps)                  bf16  (scalar engine)
  last 4 pairs: e *= mk[j,m]         (DVE tensor_mul)
  acc[:, slot] += e                  fp16  (DVE)
  po0 += v[u][:,0:128].T @ e_half;  po1 += v[u][:,128:256].T @ e_half
Outputs: outT bf16 [256,2048] (unnormalized), acc fp16 [128,4096].
Host: r = colsum(acc) folded over pair halves; out = (outT/r).T.
"""

import sys

sys.path.insert(0, "/opt/trn_rl_repo")

import numpy as np
import ml_dtypes

B, S, DM, DQ = 4, 4096, 256, 64
T = [32, 24, 16, 8]  # padded template: key-blocks per slot
STRIPES_A = [7, 5, 2, 0]  # R_A = [32, 24, 12, 4]
STRIPES_B = [6, 4, 3, 1]  # R_B = [28, 20, 16, 8]
# Per-pair mask thresholds for the last 4 pairs of a slot.
# exact slot (R == T): pairs are [ones, ones, tri(0/128), tri(256/384)]
# padded slot (R == T-4): pairs are [tri(0/128), tri(256/384), zero, zero]
TH_EXACT = [-1e9, -1e9, 0.0, 256.0]
TH_PAD = [0.0, 256.0, 1e9, 1e9]

_CACHE = {}


def _build_nc():
    import concourse.bass as bass  # noqa: F401
    import concourse.tile as tile
    from concourse import bacc, mybir

    dt = mybir.dt
    f32, bf, f16 = dt.float32, dt.bfloat16, dt.float16

    nc = bacc.Bacc(
        "TRN2",
        target_bir_lowering=False,
        debug=False,
        enable_asserts=False,
        num_devices=8,
    )

    def din(name, shape, d):
        return nc.dram_tensor(name, shape, d, kind="ExternalInput").ap()

    eq = din("eq", [256, 2048], bf)
    ek = din("ek", [256, 4096], bf)  # column-reordered: even blocks, then odd
    ev = din("ev", [256, 4096], bf)
    wq = din("wq", [256, 128], bf)  # Wq.T duplicated along cols
    wk = din("wk", [256, 64], bf)  # Wk.T
    wv = din("wv", [256, 256], bf)  # Wv.T
    th = din("th", [128, 16], f32)  # mask thresholds per (slot, pair)
    outT = nc.dram_tensor("outT", [256, 2048], bf, kind="ExternalOutput").ap()
    acc_out = nc.dram_tensor("acc", [128, 4096], f16, kind="ExternalOutput").ap()

    Exp = mybir.ActivationFunctionType.Exp
    GE = mybir.AluOpType.is_ge
    MUL = mybir.AluOpType.mult

    with tile.TileContext(nc) as tc:
        from contextlib import ExitStack

        with ExitStack() as ctx:
            const = ctx.enter_context(tc.tile_pool(name="const", bufs=1))

            # ---- persistent SBUF tensors ----
            eq_sb = [const.tile([128, 2048], bf, tag=f"eq{h}", name=f"eq{h}") for h in range(2)]
            ek_sb = [const.tile([128, 4096], bf, tag=f"ek{h}", name=f"ek{h}") for h in range(2)]
            ev_sb = [const.tile([128, 4096], bf, tag=f"ev{h}", name=f"ev{h}") for h in range(2)]
            wq_sb = const.tile([128, 256], bf, tag="wq", name="wq")
            wk_sb = const.tile([128, 128], bf, tag="wk", name="wk")
            wv_sb = const.tile([128, 512], bf, tag="wv", name="wv")
            th_sb = const.tile([128, 16], f32, tag="th", name="th")
            qT = const.tile([128, 2048], bf, tag="qT", name="qT")  # dup halves
            kTp = const.tile([128, 2048], bf, tag="kTp", name="kTp")  # pair-packed
            v_sb = const.tile([128, 32 * 256], bf, tag="v", name="v")
            qk = const.tile([128, 1024], f16, tag="qk", name="qk")
            acc = const.tile([128, 4096], f16, tag="acc", name="acc")
            mk = const.tile([128, 16 * 1024], bf, tag="mk", name="mk")

            # Input DMAs are issued from both SP and Activation queues (half
            # each) and staged per phase: phases 0-1 up front, later phases
            # from inside the pipeline so issue time hides under compute.
            def dma_phase(p, j):
                # Phase 0 splits issues across SP and ACT queues (shorter
                # critical path); later phases go all-SP so ACT stays free
                # for the exp stream.
                alt = nc.scalar if p == 0 else nc.sync
                cs_q = slice(j * 512, (j + 1) * 512)
                nc.sync.dma_start(eq_sb[0][:, cs_q], eq[0:128, cs_q])
                alt.dma_start(eq_sb[1][:, cs_q], eq[128:256, cs_q])
                if p == 0:
                    for h in range(2):
                        nc.sync.dma_start(
                            wq_sb[:, h * 128 : (h + 1) * 128], wq[h * 128 : (h + 1) * 128, :]
                        )
                        nc.sync.dma_start(
                            wk_sb[:, h * 64 : (h + 1) * 64], wk[h * 128 : (h + 1) * 128, :]
                        )
                        nc.scalar.dma_start(
                            wv_sb[:, h * 256 : (h + 1) * 256], wv[h * 128 : (h + 1) * 128, :]
                        )
                    nc.scalar.dma_start(th_sb[:], th[:])
                for reg in range(2):  # 0: even region, 1: odd region
                    cs = slice(reg * 2048 + p * 512, reg * 2048 + (p + 1) * 512)
                    nc.sync.dma_start(ek_sb[0][:, cs], ek[0:128, cs])
                    alt.dma_start(ek_sb[1][:, cs], ek[128:256, cs])
                for cc in EV_CH[p]:
                    cs = slice(cc * 512, (cc + 1) * 512)
                    nc.sync.dma_start(ev_sb[0][:, cs], ev[0:128, cs])
                    alt.dma_start(ev_sb[1][:, cs], ev[128:256, cs])

            PHASES = ((0, 3), (1, 2), (2, 1), (3, 0))
            V_QUOTA = ((0, 1, 2, 3, 4, 5), (6, 7, 8, 9), (10, 11, 12, 13), (14, 15))
            EV_CH = ((0, 1, 2), (3, 4), (5, 6), (7,))
            dma_phase(0, 3)
            dma_phase(1, 2)

            # Warm-up: dummy matmuls on scratch SBUF keep the PE busy while
            # input DMAs land, so the HAM clock-gate opens (1.2 -> 2.4 GHz)
            # before real work starts.
            scr = const.tile([128, 512], bf, tag="scr", name="scr")
            nc.gpsimd.memset(scr[:], 0.0)

            # QK[p, i2*512 + qi] = qi - 128*i2 - p
            nc.gpsimd.iota(
                qk[:],
                [[-128, 2], [1, 512]],
                base=0,
                channel_multiplier=-1,
                allow_small_or_imprecise_dtypes=True,
            )

            pp = ctx.enter_context(tc.tile_pool(name="pp", bufs=2, space="PSUM"))
            psc = ctx.enter_context(tc.tile_pool(name="psc", bufs=2, space="PSUM"))
            po_pool = ctx.enter_context(tc.tile_pool(name="po", bufs=1, space="PSUM"))
            epool = ctx.enter_context(tc.tile_pool(name="e", bufs=8))
            opool = ctx.enter_context(tc.tile_pool(name="o", bufs=2))

            for _ in range(6):
                ps = pp.tile([128, 512], f32, tag="ps", name="ps")
                nc.tensor.matmul(ps[:], scr[:, 0:128], scr[:], start=True, stop=True)

            LAG = 3
            # Phase p: projections for slot j=(3,2,1,0)[p], then attention on
            # slot j. Each phase's projections cover exactly the new kTp/v
            # columns that slot needs, so DMA/proj/attention pipeline.
            for p, j in PHASES:
                # qT chunk j (duplicated onto both halves by the dup'd wq)
                ps = pp.tile([128, 512], f32, tag="ps", name="ps")
                for h in range(2):
                    nc.tensor.matmul(
                        ps[:],
                        wq_sb[:, h * 128 : (h + 1) * 128],
                        eq_sb[h][:, j * 512 : (j + 1) * 512],
                        start=(h == 0),
                        stop=(h == 1),
                    )
                nc.vector.tensor_copy(qT[:, j * 512 : (j + 1) * 512], ps[:])
                # kTp chunk p: even blocks -> partitions 0:64, odd -> 64:128
                ps = pp.tile([128, 512], f32, tag="ps", name="ps")
                for half in range(2):
                    dst = ps[half * 64 : (half + 1) * 64, :]
                    for h in range(2):
                        nc.tensor.matmul(
                            dst,
                            wk_sb[:, h * 64 : (h + 1) * 64],
                            ek_sb[h][:, half * 2048 + p * 512 : half * 2048 + (p + 1) * 512],
                            start=(h == 0),
                            stop=(h == 1),
                        )
                nc.vector.tensor_copy(kTp[:, p * 512 : (p + 1) * 512], ps[:])
                # mask tiles for this slot: mk[t] = (QK >= th[t]) in bf16
                for m in range(4):
                    t = j * 4 + m
                    nc.vector.tensor_scalar(
                        mk[:, t * 1024 : (t + 1) * 1024],
                        qk[:],
                        th_sb[:, t : t + 1],
                        None,
                        GE,
                    )

                # ---- attention slot j ----
                npairs = T[j] // 2
                po0 = po_pool.tile([128, 512], f32, tag="po0", name="po0")
                po1 = po_pool.tile([128, 512], f32, tag="po1", name="po1")
                qs_top = qT[0:64, j * 512 : (j + 1) * 512]
                qs_bot = qT[64:128, j * 512 : (j + 1) * 512]
                acc_j = acc[:, j * 1024 : (j + 1) * 1024]
                es = [None] * npairs
                for d in range(npairs + LAG):
                    if d < npairs:
                        ps = psc.tile([128, 1024], f32, tag="ps", name="ps")
                        nc.tensor.matmul(
                            ps[:, 0:512],
                            kTp[0:64, d * 128 : (d + 1) * 128],
                            qs_top,
                            start=True,
                            stop=True,
                        )
                        nc.tensor.matmul(
                            ps[:, 512:1024],
                            kTp[64:128, d * 128 : (d + 1) * 128],
                            qs_bot,
                            start=True,
                            stop=True,
                        )
                        e = epool.tile([128, 1024], bf, tag="e", name="e")
                        nc.scalar.activation(e[:], ps[:], Exp, scale=0.125)
                        if d >= npairs - 4:
                            m = d - (npairs - 4)
                            t = j * 4 + m
                            nc.vector.tensor_mul(
                                e[:], e[:], mk[:, t * 1024 : (t + 1) * 1024]
                            )
                        if d == 0:
                            nc.vector.tensor_copy(acc_j, e[:])
                        else:
                            nc.vector.tensor_add(acc_j, acc_j, e[:])
                        es[d] = e
                        if d == 0:
                            # v-projection quota emitted after the first
                            # scores pair: the exp/mask/acc pipeline fills
                            # while the PE does these (pv needs v pair dd
                            # only at iteration dd+LAG).
                            for i in V_QUOTA[p]:
                                vps = pp.tile([128, 512], f32, tag="ps", name="ps")
                                for s in range(2):
                                    t = 2 * i + s
                                    for h in range(2):
                                        nc.tensor.matmul(
                                            vps[:, s * 256 : (s + 1) * 256],
                                            ev_sb[h][:, t * 128 : (t + 1) * 128],
                                            wv_sb[:, h * 256 : (h + 1) * 256],
                                            start=(h == 0),
                                            stop=(h == 1),
                                        )
                                nc.vector.tensor_copy(
                                    v_sb[:, i * 512 : (i + 1) * 512], vps[:]
                                )
                    if d >= LAG:
                        dd = d - LAG
                        e = es[dd]
                        for half in range(2):
                            u = 2 * dd + half
                            eh = e[:, half * 512 : (half + 1) * 512]
                            first = dd == 0 and half == 0
                            last = dd == npairs - 1 and half == 1
                            nc.tensor.matmul(
                                po0[:],
                                v_sb[:, u * 256 : u * 256 + 128],
                                eh,
                                start=first,
                                stop=last,
                            )
                            nc.tensor.matmul(
                                po1[:],
                                v_sb[:, u * 256 + 128 : (u + 1) * 256],
                                eh,
                                start=first,
                                stop=last,
                            )
                nc.sync.dma_start(acc_out[:, j * 1024 : (j + 1) * 1024], acc_j)
                o0 = opool.tile([128, 512], bf, tag="o0", name="o0")
                o1 = opool.tile([128, 512], bf, tag="o1", name="o1")
                nc.scalar.copy(o0[:], po0[:])
                if p == 3:  # final slot: parallelize the two output copies
                    nc.vector.tensor_copy(o1[:], po1[:])
                else:
                    nc.scalar.copy(o1[:], po1[:])
                nc.sync.dma_start(outT[0:128, j * 512 : (j + 1) * 512], o0[:])
                nc.sync.dma_start(outT[128:256, j * 512 : (j + 1) * 512], o1[:])
                if p + 2 <= 3:
                    dma_phase(*PHASES[p + 2])

    nc.compile()
    return nc


def kernel(encodings_for_q, encodings_for_k, encodings_for_v, mask, Wq, Wk, Wv):
    from concourse.bass_utils import run_bass_kernel_spmd

    if "nc" not in _CACHE:
        _CACHE["nc"] = _build_nc()
    nc = _CACHE["nc"]

    bf = ml_dtypes.bfloat16
    wqd = np.ascontiguousarray(
        np.concatenate([Wq.T, Wq.T], axis=1), dtype=bf
    )  # [256,128]
    wkt = np.ascontiguousarray(Wk.T, dtype=bf)  # [256,64]
    wvt = np.ascontiguousarray(Wv.T, dtype=bf)  # [256,256]

    in_maps = []
    metas = []
    for c in range(8):
        b, t = c // 2, c % 2
        stripes = STRIPES_A if t == 0 else STRIPES_B
        eqT = np.concatenate(
            [encodings_for_q[b, st * 512 : (st + 1) * 512, :].T for st in stripes],
            axis=1,
        )
        ekT = encodings_for_k[b].T.reshape(256, 32, 128)
        ek_reord = np.concatenate([ekT[:, 0::2, :], ekT[:, 1::2, :]], axis=1).reshape(
            256, 4096
        )
        # thresholds: slot j exact if R[j] == T[j]
        thr = np.empty((16,), dtype=np.float32)
        for j in range(4):
            R = 4 * (stripes[j] + 1)
            vals = TH_EXACT if R == T[j] else TH_PAD
            thr[j * 4 : (j + 1) * 4] = vals
        in_maps.append(
            {
                "eq": np.ascontiguousarray(eqT, dtype=bf),
                "ek": np.ascontiguousarray(ek_reord, dtype=bf),
                "ev": np.ascontiguousarray(encodings_for_v[b].T, dtype=bf),
                "wq": wqd,
                "wk": wkt,
                "wv": wvt,
                "th": np.ascontiguousarray(np.broadcast_to(thr, (128, 16))),
            }
        )
        metas.append((b, stripes))

    res = run_bass_kernel_spmd(nc, in_maps, core_ids=list(range(8)))
    _CACHE["last_res"] = res

    out = np.empty((B, S, DM), dtype=np.float32)
    for c in range(8):
        b, stripes = metas[c]
        oT = res.results[c]["outT"].astype(np.float32)
        a = res.results[c]["acc"].astype(np.float32)
        for j, st in enumerate(stripes):
            r = a[:, j * 1024 : j * 1024 + 512].sum(0) + a[
                :, j * 1024 + 512 : (j + 1) * 1024
            ].sum(0)
            blk = oT[:, j * 512 : (j + 1) * 512] / r[None, :]
            out[b, st * 512 : (st + 1) * 512, :] = blk.T
    return out


# revision 51
# speedup vs baseline: 1.1303x; 1.1303x over previous
"""Causal attention head on 8 trn2 NeuronCores.

Sharding: core c = (batch b = c//2, type t = c%2). Each core handles 4
query stripes of 512 of its batch. Causal balance: type A gets stripes
[7,5,2,0] with real key-block counts R_A=[32,24,12,4]; type B stripes
[6,4,3,1] with R_B=[28,20,16,8]. One SPMD program: every core runs the
padded template T=[32,24,16,8]; per-core behaviour comes only from input
data (per-core threshold scalars select ones/triangle/zero mask tiles).

Everything on the PE array is bf16 (fp32 HIGH poisons fast-weight-load
and fp32 moving operands stream at half rate). Score matmuls pack two
key-blocks per issue via PE row tiling: kT pairs live on partition
halves 0:64 / 64:128 (host interleaves ek into even/odd block regions),
qT is duplicated onto both halves by a column-duplicated Wq.

The program is a 4-phase pipeline, one phase per slot (processed small
to large: j=3,2,1,0). Phase p: DMA group for phase p+2, projections for
just the new kTp chunk / qT chunk / a quota of v blocks this slot needs,
4 mask tiles mk[t]=(QK>=th[t]) (QK is a gpsimd iota qi-kp-128*i2; th in
{-1e9,0,256,1e9} selects ones/triangle/zero), then the slot's attention.
Input DMAs are split across the SP and ACT issue queues; a handful of
dummy matmuls at the start keep the PE busy under the DMA wait so the
HAM clock-gate opens (1.2->2.4 GHz) before real work.

Per pair d (= template positions 2d, 2d+1) of slot j:
  ps[:,   0: 512] = kTp[ 0: 64, d].T @ qT[ 0: 64, slot]   (rows 0-63)
  ps[:, 512:1024] = kTp[64:128, d].T @ qT[64:128, slot]   (rows 64-127)
  e = exp(0.125*ps)                  bf16  (scalar engine)
  last 4 pairs: e *= mk[j,m]         (DVE tensor_mul)
  acc[:, slot] += e                  fp16  (DVE)
  po0 += v[u][:,0:128].T @ e_half;  po1 += v[u][:,128:256].T @ e_half
Outputs: outT bf16 [256,2048] (unnormalized), acc fp16 [128,4096].
Host: r = colsum(acc) folded over pair halves; out = (outT/r).T.
"""

import sys

sys.path.insert(0, "/opt/trn_rl_repo")

import numpy as np
import ml_dtypes

B, S, DM, DQ = 4, 4096, 256, 64
T = [32, 24, 16, 8]  # padded template: key-blocks per slot
STRIPES_A = [7, 5, 2, 0]  # R_A = [32, 24, 12, 4]
STRIPES_B = [6, 4, 3, 1]  # R_B = [28, 20, 16, 8]
# Per-pair mask thresholds for the last 4 pairs of a slot.
# exact slot (R == T): pairs are [ones, ones, tri(0/128), tri(256/384)]
# padded slot (R == T-4): pairs are [tri(0/128), tri(256/384), zero, zero]
TH_EXACT = [-1e9, -1e9, 0.0, 256.0]
TH_PAD = [0.0, 256.0, 1e9, 1e9]

_CACHE = {}


def _build_nc():
    import concourse.bass as bass  # noqa: F401
    import concourse.tile as tile
    from concourse import bacc, mybir

    dt = mybir.dt
    f32, bf, f16 = dt.float32, dt.bfloat16, dt.float16

    nc = bacc.Bacc(
        "TRN2",
        target_bir_lowering=False,
        debug=False,
        enable_asserts=False,
        num_devices=8,
    )

    def din(name, shape, d):
        return nc.dram_tensor(name, shape, d, kind="ExternalInput").ap()

    eq = din("eq", [256, 2048], bf)
    ek = din("ek", [256, 4096], bf)  # column-reordered: even blocks, then odd
    ev = din("ev", [256, 4096], bf)
    wq = din("wq", [256, 128], bf)  # Wq.T duplicated along cols
    wk = din("wk", [256, 64], bf)  # Wk.T
    wv = din("wv", [256, 256], bf)  # Wv.T
    th = din("th", [128, 16], f32)  # mask thresholds per (slot, pair)
    outT = nc.dram_tensor("outT", [256, 2048], bf, kind="ExternalOutput").ap()
    acc_out = nc.dram_tensor("acc", [128, 4096], f16, kind="ExternalOutput").ap()

    Exp = mybir.ActivationFunctionType.Exp
    GE = mybir.AluOpType.is_ge
    MUL = mybir.AluOpType.mult

    with tile.TileContext(nc) as tc:
        from contextlib import ExitStack

        with ExitStack() as ctx:
            const = ctx.enter_context(tc.tile_pool(name="const", bufs=1))

            # ---- persistent SBUF tensors ----
            eq_sb = [const.tile([128, 2048], bf, tag=f"eq{h}", name=f"eq{h}") for h in range(2)]
            ek_sb = [const.tile([128, 4096], bf, tag=f"ek{h}", name=f"ek{h}") for h in range(2)]
            ev_sb = [const.tile([128, 4096], bf, tag=f"ev{h}", name=f"ev{h}") for h in range(2)]
            wq_sb = const.tile([128, 256], bf, tag="wq", name="wq")
            wk_sb = const.tile([128, 128], bf, tag="wk", name="wk")
            wv_sb = const.tile([128, 512], bf, tag="wv", name="wv")
            th_sb = const.tile([128, 16], f32, tag="th", name="th")
            qT = const.tile([128, 2048], bf, tag="qT", name="qT")  # dup halves
            kTp = const.tile([128, 2048], bf, tag="kTp", name="kTp")  # pair-packed
            v_sb = const.tile([128, 32 * 256], bf, tag="v", name="v")
            qk = const.tile([128, 1024], f16, tag="qk", name="qk")
            acc = const.tile([128, 4096], f16, tag="acc", name="acc")
            mk = const.tile([128, 16 * 1024], bf, tag="mk", name="mk")

            # Input DMAs are issued from both SP and Activation queues (half
            # each) and staged per phase: phases 0-1 up front, later phases
            # from inside the pipeline so issue time hides under compute.
            def dma_phase(p, j):
                # Phase 0 splits issues across SP and ACT queues (shorter
                # critical path); later phases go all-SP so ACT stays free
                # for the exp stream.
                alt = nc.scalar if p == 0 else nc.sync
                cs_q = slice(j * 512, (j + 1) * 512)
                nc.sync.dma_start(eq_sb[0][:, cs_q], eq[0:128, cs_q])
                alt.dma_start(eq_sb[1][:, cs_q], eq[128:256, cs_q])
                if p == 0:
                    for h in range(2):
                        nc.sync.dma_start(
                            wq_sb[:, h * 128 : (h + 1) * 128], wq[h * 128 : (h + 1) * 128, :]
                        )
                        nc.sync.dma_start(
                            wk_sb[:, h * 64 : (h + 1) * 64], wk[h * 128 : (h + 1) * 128, :]
                        )
                        nc.scalar.dma_start(
                            wv_sb[:, h * 256 : (h + 1) * 256], wv[h * 128 : (h + 1) * 128, :]
                        )
                    nc.scalar.dma_start(th_sb[:], th[:])
                for reg in range(2):  # 0: even region, 1: odd region
                    cs = slice(reg * 2048 + p * 512, reg * 2048 + (p + 1) * 512)
                    nc.sync.dma_start(ek_sb[0][:, cs], ek[0:128, cs])
                    alt.dma_start(ek_sb[1][:, cs], ek[128:256, cs])
                for cc in EV_CH[p]:
                    cs = slice(cc * 512, (cc + 1) * 512)
                    nc.sync.dma_start(ev_sb[0][:, cs], ev[0:128, cs])
                    alt.dma_start(ev_sb[1][:, cs], ev[128:256, cs])

            PHASES = ((0, 3), (1, 2), (2, 1), (3, 0))
            V_QUOTA = ((0, 1, 2, 3, 4, 5), (6, 7, 8, 9), (10, 11, 12, 13), (14, 15))
            EV_CH = ((0, 1, 2), (3, 4), (5, 6), (7,))
            dma_phase(0, 3)
            dma_phase(1, 2)

            # Warm-up: dummy matmuls on scratch SBUF keep the PE busy while
            # input DMAs land, so the HAM clock-gate opens (1.2 -> 2.4 GHz)
            # before real work starts.
            scr = const.tile([128, 512], bf, tag="scr", name="scr")
            nc.gpsimd.memset(scr[:], 0.0)

            # QK[p, i2*512 + qi] = qi - 128*i2 - p
            nc.gpsimd.iota(
                qk[:],
                [[-128, 2], [1, 512]],
                base=0,
                channel_multiplier=-1,
                allow_small_or_imprecise_dtypes=True,
            )

            pp = ctx.enter_context(tc.tile_pool(name="pp", bufs=2, space="PSUM"))
            psc = ctx.enter_context(tc.tile_pool(name="psc", bufs=2, space="PSUM"))
            po_pool = ctx.enter_context(tc.tile_pool(name="po", bufs=1, space="PSUM"))
            epool = ctx.enter_context(tc.tile_pool(name="e", bufs=8))
            opool = ctx.enter_context(tc.tile_pool(name="o", bufs=2))

            for _ in range(6):
                ps = pp.tile([128, 512], f32, tag="ps", name="ps")
                nc.tensor.matmul(ps[:], scr[:, 0:128], scr[:], start=True, stop=True)

            LAG = 3
            # Phase p: projections for slot j=(3,2,1,0)[p], then attention on
            # slot j. Each phase's projections cover exactly the new kTp/v
            # columns that slot needs, so DMA/proj/attention pipeline.
            for p, j in PHASES:
                # qT chunk j (duplicated onto both halves by the dup'd wq)
                ps = pp.tile([128, 512], f32, tag="ps", name="ps")
                for h in range(2):
                    nc.tensor.matmul(
                        ps[:],
                        wq_sb[:, h * 128 : (h + 1) * 128],
                        eq_sb[h][:, j * 512 : (j + 1) * 512],
                        start=(h == 0),
                        stop=(h == 1),
                    )
                nc.vector.tensor_copy(qT[:, j * 512 : (j + 1) * 512], ps[:])
                # kTp chunk p: even blocks -> partitions 0:64, odd -> 64:128
                ps = pp.tile([128, 512], f32, tag="ps", name="ps")
                for half in range(2):
                    dst = ps[half * 64 : (half + 1) * 64, :]
                    for h in range(2):
                        nc.tensor.matmul(
                            dst,
                            wk_sb[:, h * 64 : (h + 1) * 64],
                            ek_sb[h][:, half * 2048 + p * 512 : half * 2048 + (p + 1) * 512],
                            start=(h == 0),
                            stop=(h == 1),
                        )
                nc.vector.tensor_copy(kTp[:, p * 512 : (p + 1) * 512], ps[:])
                # v pairs for this phase (natural [keys, 256] bf16)
                for i in V_QUOTA[p]:
                    ps = pp.tile([128, 512], f32, tag="ps", name="ps")
                    for s in range(2):
                        t = 2 * i + s
                        for h in range(2):
                            nc.tensor.matmul(
                                ps[:, s * 256 : (s + 1) * 256],
                                ev_sb[h][:, t * 128 : (t + 1) * 128],
                                wv_sb[:, h * 256 : (h + 1) * 256],
                                start=(h == 0),
                                stop=(h == 1),
                            )
                    nc.vector.tensor_copy(v_sb[:, i * 512 : (i + 1) * 512], ps[:])
                # mask tiles for this slot: mk[t] = (QK >= th[t]) in bf16
                for m in range(4):
                    t = j * 4 + m
                    nc.vector.tensor_scalar(
                        mk[:, t * 1024 : (t + 1) * 1024],
                        qk[:],
                        th_sb[:, t : t + 1],
                        None,
                        GE,
                    )

                # ---- attention slot j ----
                npairs = T[j] // 2
                po0 = po_pool.tile([128, 512], f32, tag="po0", name="po0")
                po1 = po_pool.tile([128, 512], f32, tag="po1", name="po1")
                qs_top = qT[0:64, j * 512 : (j + 1) * 512]
                qs_bot = qT[64:128, j * 512 : (j + 1) * 512]
                acc_j = acc[:, j * 1024 : (j + 1) * 1024]
                es = [None] * npairs
                for d in range(npairs + LAG):
                    if d < npairs:
                        ps = psc.tile([128, 1024], f32, tag="ps", name="ps")
                        nc.tensor.matmul(
                            ps[:, 0:512],
                            kTp[0:64, d * 128 : (d + 1) * 128],
                            qs_top,
                            start=True,
                            stop=True,
                        )
                        nc.tensor.matmul(
                            ps[:, 512:1024],
                            kTp[64:128, d * 128 : (d + 1) * 128],
                            qs_bot,
                            start=True,
                            stop=True,
                        )
                        e = epool.tile([128, 1024], bf, tag="e", name="e")
                        nc.scalar.activation(e[:], ps[:], Exp, scale=0.125)
                        if d >= npairs - 4:
                            m = d - (npairs - 4)
                            t = j * 4 + m
                            nc.vector.tensor_mul(
                                e[:], e[:], mk[:, t * 1024 : (t + 1) * 1024]
                            )
                        if d == 0:
                            nc.vector.tensor_copy(acc_j, e[:])
                        else:
                            nc.vector.tensor_add(acc_j, acc_j, e[:])
                        es[d] = e
                    if d >= LAG:
                        dd = d - LAG
                        e = es[dd]
                        for half in range(2):
                            u = 2 * dd + half
                            eh = e[:, half * 512 : (half + 1) * 512]
                            first = dd == 0 and half == 0
                            last = dd == npairs - 1 and half == 1
                            nc.tensor.matmul(
                                po0[:],
                                v_sb[:, u * 256 : u * 256 + 128],
                                eh,
                                start=first,
                                stop=last,
                            )
                            nc.tensor.matmul(
                                po1[:],
                                v_sb[:, u * 256 + 128 : (u + 1) * 256],
                                eh,
                                start=first,
                                stop=last,
                            )
                nc.sync.dma_start(acc_out[:, j * 1024 : (j + 1) * 1024], acc_j)
                o0 = opool.tile([128, 512], bf, tag="o0", name="o0")
                o1 = opool.tile([128, 512], bf, tag="o1", name="o1")
                nc.scalar.copy(o0[:], po0[:])
                if p == 3:  # final slot: parallelize the two output copies
                    nc.vector.tensor_copy(o1[:], po1[:])
                else:
                    nc.scalar.copy(o1[:], po1[:])
                nc.sync.dma_start(outT[0:128, j * 512 : (j + 1) * 512], o0[:])
                nc.sync.dma_start(outT[128:256, j * 512 : (j + 1) * 512], o1[:])
                if p + 2 <= 3:
                    dma_phase(*PHASES[p + 2])

    nc.compile()
    return nc


def kernel(encodings_for_q, encodings_for_k, encodings_for_v, mask, Wq, Wk, Wv):
    from concourse.bass_utils import run_bass_kernel_spmd

    if "nc" not in _CACHE:
        _CACHE["nc"] = _build_nc()
    nc = _CACHE["nc"]

    bf = ml_dtypes.bfloat16
    wqd = np.ascontiguousarray(
        np.concatenate([Wq.T, Wq.T], axis=1), dtype=bf
    )  # [256,128]
    wkt = np.ascontiguousarray(Wk.T, dtype=bf)  # [256,64]
    wvt = np.ascontiguousarray(Wv.T, dtype=bf)  # [256,256]

    in_maps = []
    metas = []
    for c in range(8):
        b, t = c // 2, c % 2
        stripes = STRIPES_A if t == 0 else STRIPES_B
        eqT = np.concatenate(
            [encodings_for_q[b, st * 512 : (st + 1) * 512, :].T for st in stripes],
            axis=1,
        )
        ekT = encodings_for_k[b].T.reshape(256, 32, 128)
        ek_reord = np.concatenate([ekT[:, 0::2, :], ekT[:, 1::2, :]], axis=1).reshape(
            256, 4096
        )
        # thresholds: slot j exact if R[j] == T[j]
        thr = np.empty((16,), dtype=np.float32)
        for j in range(4):
            R = 4 * (stripes[j] + 1)
            vals = TH_EXACT if R == T[j] else TH_PAD
            thr[j * 4 : (j + 1) * 4] = vals
        in_maps.append(
            {
                "eq": np.ascontiguousarray(eqT, dtype=bf),
                "ek": np.ascontiguousarray(ek_reord, dtype=bf),
                "ev": np.ascontiguousarray(encodings_for_v[b].T, dtype=bf),
                "wq": wqd,
                "wk": wkt,
                "wv": wvt,
                "th": np.ascontiguousarray(np.broadcast_to(thr, (128, 16))),
            }
        )
        metas.append((b, stripes))

    res = run_bass_kernel_spmd(nc, in_maps, core_ids=list(range(8)))
    _CACHE["last_res"] = res

    out = np.empty((B, S, DM), dtype=np.float32)
    for c in range(8):
        b, stripes = metas[c]
        oT = res.results[c]["outT"].astype(np.float32)
        a = res.results[c]["acc"].astype(np.float32)
        for j, st in enumerate(stripes):
            r = a[:, j * 1024 : j * 1024 + 512].sum(0) + a[
                :, j * 1024 + 512 : (j + 1) * 1024
            ].sum(0)
            blk = oT[:, j * 512 : (j + 1) * 512] / r[None, :]
            out[b, st * 512 : (st + 1) * 512, :] = blk.T
    return out


# revision 52
# speedup vs baseline: 1.1418x; 1.0101x over previous
"""Causal attention head on 8 trn2 NeuronCores.

Sharding: core c = (batch b = c//2, type t = c%2). Each core handles 4
query stripes of 512 of its batch. Causal balance: type A gets stripes
[7,5,2,0] with real key-block counts R_A=[32,24,12,4]; type B stripes
[6,4,3,1] with R_B=[28,20,16,8]. One SPMD program: every core runs the
padded template T=[32,24,16,8]; per-core behaviour comes only from input
data (per-core threshold scalars select ones/triangle/zero mask tiles).

Everything on the PE array is bf16 (fp32 HIGH poisons fast-weight-load
and fp32 moving operands stream at half rate). Score matmuls pack two
key-blocks per issue via PE row tiling: kT pairs live on partition
halves 0:64 / 64:128 (host interleaves ek into even/odd block regions),
qT is duplicated onto both halves by a column-duplicated Wq.

The program is a 4-phase pipeline, one phase per slot (processed small
to large: j=3,2,1,0). Phase p: DMA group for phase p+2, projections for
just the new kTp chunk / qT chunk / a quota of v blocks this slot needs,
4 mask tiles mk[t]=(QK>=th[t]) (QK is a gpsimd iota qi-kp-128*i2; th in
{-1e9,0,256,1e9} selects ones/triangle/zero), then the slot's attention.
Input DMAs are split across the SP and ACT issue queues; a handful of
dummy matmuls at the start keep the PE busy under the DMA wait so the
HAM clock-gate opens (1.2->2.4 GHz) before real work.

Per pair d (= template positions 2d, 2d+1) of slot j:
  ps[:,   0: 512] = kTp[ 0: 64, d].T @ qT[ 0: 64, slot]   (rows 0-63)
  ps[:, 512:1024] = kTp[64:128, d].T @ qT[64:128, slot]   (rows 64-127)
  e = exp(0.125*ps)                  bf16  (scalar engine)
  last 4 pairs: e *= mk[j,m]         (DVE tensor_mul)
  acc[:, slot] += e                  fp16  (DVE)
  po0 += v[u][:,0:128].T @ e_half;  po1 += v[u][:,128:256].T @ e_half
Outputs: outT bf16 [256,2048] (unnormalized), acc fp16 [128,4096].
Host: r = colsum(acc) folded over pair halves; out = (outT/r).T.
"""

import sys

sys.path.insert(0, "/opt/trn_rl_repo")

import numpy as np
import ml_dtypes

B, S, DM, DQ = 4, 4096, 256, 64
T = [32, 24, 16, 8]  # padded template: key-blocks per slot
STRIPES_A = [7, 5, 2, 0]  # R_A = [32, 24, 12, 4]
STRIPES_B = [6, 4, 3, 1]  # R_B = [28, 20, 16, 8]
# Per-pair mask thresholds for the last 4 pairs of a slot.
# exact slot (R == T): pairs are [ones, ones, tri(0/128), tri(256/384)]
# padded slot (R == T-4): pairs are [tri(0/128), tri(256/384), zero, zero]
TH_EXACT = [-1e9, -1e9, 0.0, 256.0]
TH_PAD = [0.0, 256.0, 1e9, 1e9]

_CACHE = {}


def _build_nc():
    import concourse.bass as bass  # noqa: F401
    import concourse.tile as tile
    from concourse import bacc, mybir

    dt = mybir.dt
    f32, bf, f16 = dt.float32, dt.bfloat16, dt.float16

    nc = bacc.Bacc(
        "TRN2",
        target_bir_lowering=False,
        debug=False,
        enable_asserts=False,
        num_devices=8,
    )

    def din(name, shape, d):
        return nc.dram_tensor(name, shape, d, kind="ExternalInput").ap()

    eq = din("eq", [256, 2048], bf)
    ek = din("ek", [256, 4096], bf)  # column-reordered: even blocks, then odd
    ev = din("ev", [256, 4096], bf)
    wq = din("wq", [256, 128], bf)  # Wq.T duplicated along cols
    wk = din("wk", [256, 64], bf)  # Wk.T
    wv = din("wv", [256, 256], bf)  # Wv.T
    th = din("th", [128, 16], f32)  # mask thresholds per (slot, pair)
    outT = nc.dram_tensor("outT", [256, 2048], bf, kind="ExternalOutput").ap()
    acc_out = nc.dram_tensor("acc", [128, 4096], f16, kind="ExternalOutput").ap()

    Exp = mybir.ActivationFunctionType.Exp
    GE = mybir.AluOpType.is_ge
    MUL = mybir.AluOpType.mult

    with tile.TileContext(nc) as tc:
        from contextlib import ExitStack

        with ExitStack() as ctx:
            const = ctx.enter_context(tc.tile_pool(name="const", bufs=1))

            # ---- persistent SBUF tensors ----
            eq_sb = [const.tile([128, 2048], bf, tag=f"eq{h}", name=f"eq{h}") for h in range(2)]
            ek_sb = [const.tile([128, 4096], bf, tag=f"ek{h}", name=f"ek{h}") for h in range(2)]
            ev_sb = [const.tile([128, 4096], bf, tag=f"ev{h}", name=f"ev{h}") for h in range(2)]
            wq_sb = const.tile([128, 256], bf, tag="wq", name="wq")
            wk_sb = const.tile([128, 128], bf, tag="wk", name="wk")
            wv_sb = const.tile([128, 512], bf, tag="wv", name="wv")
            th_sb = const.tile([128, 16], f32, tag="th", name="th")
            qT = const.tile([128, 2048], bf, tag="qT", name="qT")  # dup halves
            kTp = const.tile([128, 2048], bf, tag="kTp", name="kTp")  # pair-packed
            v_sb = const.tile([128, 32 * 256], bf, tag="v", name="v")
            qk = const.tile([128, 1024], f16, tag="qk", name="qk")
            acc = const.tile([128, 4096], f16, tag="acc", name="acc")
            mk = const.tile([128, 16 * 1024], bf, tag="mk", name="mk")

            # Input DMAs are issued from both SP and Activation queues (half
            # each) and staged per phase: phases 0-1 up front, later phases
            # from inside the pipeline so issue time hides under compute.
            def dma_phase(p, j):
                # Phase 0 splits issues across SP and ACT queues (shorter
                # critical path); later phases go all-SP so ACT stays free
                # for the exp stream.
                alt = nc.scalar if p == 0 else nc.sync
                cs_q = slice(j * 512, (j + 1) * 512)
                nc.sync.dma_start(eq_sb[0][:, cs_q], eq[0:128, cs_q])
                alt.dma_start(eq_sb[1][:, cs_q], eq[128:256, cs_q])
                if p == 0:
                    for h in range(2):
                        nc.sync.dma_start(
                            wq_sb[:, h * 128 : (h + 1) * 128], wq[h * 128 : (h + 1) * 128, :]
                        )
                        nc.sync.dma_start(
                            wk_sb[:, h * 64 : (h + 1) * 64], wk[h * 128 : (h + 1) * 128, :]
                        )
                        nc.scalar.dma_start(
                            wv_sb[:, h * 256 : (h + 1) * 256], wv[h * 128 : (h + 1) * 128, :]
                        )
                    nc.scalar.dma_start(th_sb[:], th[:])
                for reg in range(2):  # 0: even region, 1: odd region
                    cs = slice(reg * 2048 + p * 512, reg * 2048 + (p + 1) * 512)
                    nc.sync.dma_start(ek_sb[0][:, cs], ek[0:128, cs])
                    alt.dma_start(ek_sb[1][:, cs], ek[128:256, cs])
                for cc in EV_CH[p]:
                    cs = slice(cc * 512, (cc + 1) * 512)
                    nc.sync.dma_start(ev_sb[0][:, cs], ev[0:128, cs])
                    alt.dma_start(ev_sb[1][:, cs], ev[128:256, cs])

            PHASES = ((0, 3), (1, 2), (2, 1), (3, 0))
            V_QUOTA = ((0, 1, 2, 3, 4, 5), (6, 7, 8, 9), (10, 11, 12, 13), (14, 15))
            EV_CH = ((0, 1, 2), (3, 4), (5, 6), (7,))
            dma_phase(0, 3)
            dma_phase(1, 2)

            # Warm-up: dummy matmuls on scratch SBUF keep the PE busy while
            # input DMAs land, so the HAM clock-gate opens (1.2 -> 2.4 GHz)
            # before real work starts.
            scr = const.tile([128, 512], bf, tag="scr", name="scr")
            nc.gpsimd.memset(scr[:], 0.0)

            # QK[p, i2*512 + qi] = qi - 128*i2 - p
            nc.gpsimd.iota(
                qk[:],
                [[-128, 2], [1, 512]],
                base=0,
                channel_multiplier=-1,
                allow_small_or_imprecise_dtypes=True,
            )

            pp = ctx.enter_context(tc.tile_pool(name="pp", bufs=2, space="PSUM"))
            psc = ctx.enter_context(tc.tile_pool(name="psc", bufs=2, space="PSUM"))
            po_pool = ctx.enter_context(tc.tile_pool(name="po", bufs=1, space="PSUM"))
            epool = ctx.enter_context(tc.tile_pool(name="e", bufs=8))
            opool = ctx.enter_context(tc.tile_pool(name="o", bufs=2))

            for _ in range(6):
                ps = pp.tile([128, 512], f32, tag="ps", name="ps")
                nc.tensor.matmul(ps[:], scr[:, 0:128], scr[:], start=True, stop=True)

            LAG = 3
            # Phase p: projections for slot j=(3,2,1,0)[p], then attention on
            # slot j. Each phase's projections cover exactly the new kTp/v
            # columns that slot needs, so DMA/proj/attention pipeline.
            for p, j in PHASES:
                # qT chunk j (duplicated onto both halves by the dup'd wq)
                ps = pp.tile([128, 512], f32, tag="ps", name="ps")
                for h in range(2):
                    nc.tensor.matmul(
                        ps[:],
                        wq_sb[:, h * 128 : (h + 1) * 128],
                        eq_sb[h][:, j * 512 : (j + 1) * 512],
                        start=(h == 0),
                        stop=(h == 1),
                    )
                nc.vector.tensor_copy(qT[:, j * 512 : (j + 1) * 512], ps[:])
                # kTp chunk p: even blocks -> partitions 0:64, odd -> 64:128
                ps = pp.tile([128, 512], f32, tag="ps", name="ps")
                for half in range(2):
                    dst = ps[half * 64 : (half + 1) * 64, :]
                    for h in range(2):
                        nc.tensor.matmul(
                            dst,
                            wk_sb[:, h * 64 : (h + 1) * 64],
                            ek_sb[h][:, half * 2048 + p * 512 : half * 2048 + (p + 1) * 512],
                            start=(h == 0),
                            stop=(h == 1),
                        )
                nc.vector.tensor_copy(kTp[:, p * 512 : (p + 1) * 512], ps[:])
                # v pairs for this phase (natural [keys, 256] bf16); copies
                # alternate DVE/ACT so the pp psum pool recycles fast enough
                for n, i in enumerate(V_QUOTA[p]):
                    ps = pp.tile([128, 512], f32, tag="ps", name="ps")
                    for s in range(2):
                        t = 2 * i + s
                        for h in range(2):
                            nc.tensor.matmul(
                                ps[:, s * 256 : (s + 1) * 256],
                                ev_sb[h][:, t * 128 : (t + 1) * 128],
                                wv_sb[:, h * 256 : (h + 1) * 256],
                                start=(h == 0),
                                stop=(h == 1),
                            )
                    dst = v_sb[:, i * 512 : (i + 1) * 512]
                    if n % 2 == 0:
                        nc.scalar.copy(dst, ps[:])
                    else:
                        nc.vector.tensor_copy(dst, ps[:])
                # mask tiles for this slot: mk[t] = (QK >= th[t]) in bf16
                for m in range(4):
                    t = j * 4 + m
                    nc.vector.tensor_scalar(
                        mk[:, t * 1024 : (t + 1) * 1024],
                        qk[:],
                        th_sb[:, t : t + 1],
                        None,
                        GE,
                    )

                # ---- attention slot j ----
                npairs = T[j] // 2
                po0 = po_pool.tile([128, 512], f32, tag="po0", name="po0")
                po1 = po_pool.tile([128, 512], f32, tag="po1", name="po1")
                qs_top = qT[0:64, j * 512 : (j + 1) * 512]
                qs_bot = qT[64:128, j * 512 : (j + 1) * 512]
                acc_j = acc[:, j * 1024 : (j + 1) * 1024]
                es = [None] * npairs
                for d in range(npairs + LAG):
                    if d < npairs:
                        ps = psc.tile([128, 1024], f32, tag="ps", name="ps")
                        nc.tensor.matmul(
                            ps[:, 0:512],
                            kTp[0:64, d * 128 : (d + 1) * 128],
                            qs_top,
                            start=True,
                            stop=True,
                        )
                        nc.tensor.matmul(
                            ps[:, 512:1024],
                            kTp[64:128, d * 128 : (d + 1) * 128],
                            qs_bot,
                            start=True,
                            stop=True,
                        )
                        e = epool.tile([128, 1024], bf, tag="e", name="e")
                        nc.scalar.activation(e[:], ps[:], Exp, scale=0.125)
                        if d >= npairs - 4:
                            m = d - (npairs - 4)
                            t = j * 4 + m
                            nc.vector.tensor_mul(
                                e[:], e[:], mk[:, t * 1024 : (t + 1) * 1024]
                            )
                        if d == 0:
                            nc.vector.tensor_copy(acc_j, e[:])
                        else:
                            nc.vector.tensor_add(acc_j, acc_j, e[:])
                        es[d] = e
                    if d >= LAG:
                        dd = d - LAG
                        e = es[dd]
                        for half in range(2):
                            u = 2 * dd + half
                            eh = e[:, half * 512 : (half + 1) * 512]
                            first = dd == 0 and half == 0
                            last = dd == npairs - 1 and half == 1
                            nc.tensor.matmul(
                                po0[:],
                                v_sb[:, u * 256 : u * 256 + 128],
                                eh,
                                start=first,
                                stop=last,
                            )
                            nc.tensor.matmul(
                                po1[:],
                                v_sb[:, u * 256 + 128 : (u + 1) * 256],
                                eh,
                                start=first,
                                stop=last,
                            )
                nc.sync.dma_start(acc_out[:, j * 1024 : (j + 1) * 1024], acc_j)
                o0 = opool.tile([128, 512], bf, tag="o0", name="o0")
                o1 = opool.tile([128, 512], bf, tag="o1", name="o1")
                nc.scalar.copy(o0[:], po0[:])
                if p == 3:  # final slot: parallelize the two output copies
                    nc.vector.tensor_copy(o1[:], po1[:])
                else:
                    nc.scalar.copy(o1[:], po1[:])
                nc.sync.dma_start(outT[0:128, j * 512 : (j + 1) * 512], o0[:])
                nc.sync.dma_start(outT[128:256, j * 512 : (j + 1) * 512], o1[:])
                if p + 2 <= 3:
                    dma_phase(*PHASES[p + 2])

    nc.compile()
    return nc


def kernel(encodings_for_q, encodings_for_k, encodings_for_v, mask, Wq, Wk, Wv):
    from concourse.bass_utils import run_bass_kernel_spmd

    if "nc" not in _CACHE:
        _CACHE["nc"] = _build_nc()
    nc = _CACHE["nc"]

    bf = ml_dtypes.bfloat16
    wqd = np.ascontiguousarray(
        np.concatenate([Wq.T, Wq.T], axis=1), dtype=bf
    )  # [256,128]
    wkt = np.ascontiguousarray(Wk.T, dtype=bf)  # [256,64]
    wvt = np.ascontiguousarray(Wv.T, dtype=bf)  # [256,256]

    in_maps = []
    metas = []
    for c in range(8):
        b, t = c // 2, c % 2
        stripes = STRIPES_A if t == 0 else STRIPES_B
        eqT = np.concatenate(
            [encodings_for_q[b, st * 512 : (st + 1) * 512, :].T for st in stripes],
            axis=1,
        )
        ekT = encodings_for_k[b].T.reshape(256, 32, 128)
        ek_reord = np.concatenate([ekT[:, 0::2, :], ekT[:, 1::2, :]], axis=1).reshape(
            256, 4096
        )
        # thresholds: slot j exact if R[j] == T[j]
        thr = np.empty((16,), dtype=np.float32)
        for j in range(4):
            R = 4 * (stripes[j] + 1)
            vals = TH_EXACT if R == T[j] else TH_PAD
            thr[j * 4 : (j + 1) * 4] = vals
        in_maps.append(
            {
                "eq": np.ascontiguousarray(eqT, dtype=bf),
                "ek": np.ascontiguousarray(ek_reord, dtype=bf),
                "ev": np.ascontiguousarray(encodings_for_v[b].T, dtype=bf),
                "wq": wqd,
                "wk": wkt,
                "wv": wvt,
                "th": np.ascontiguousarray(np.broadcast_to(thr, (128, 16))),
            }
        )
        metas.append((b, stripes))

    res = run_bass_kernel_spmd(nc, in_maps, core_ids=list(range(8)))
    _CACHE["last_res"] = res

    out = np.empty((B, S, DM), dtype=np.float32)
    for c in range(8):
        b, stripes = metas[c]
        oT = res.results[c]["outT"].astype(np.float32)
        a = res.results[c]["acc"].astype(np.float32)
        for j, st in enumerate(stripes):
            r = a[:, j * 1024 : j * 1024 + 512].sum(0) + a[
                :, j * 1024 + 512 : (j + 1) * 1024
            ].sum(0)
            blk = oT[:, j * 512 : (j + 1) * 512] / r[None, :]
            out[b, st * 512 : (st + 1) * 512, :] = blk.T
    return out


# revision 53
# speedup vs baseline: 1.1590x; 1.0151x over previous
"""Causal attention head on 8 trn2 NeuronCores.

Sharding: core c = (batch b = c//2, type t = c%2). Each core handles 4
query stripes of 512 of its batch. Causal balance: type A gets stripes
[7,5,2,0] with real key-block counts R_A=[32,24,12,4]; type B stripes
[6,4,3,1] with R_B=[28,20,16,8]. One SPMD program: every core runs the
padded template T=[32,24,16,8]; per-core behaviour comes only from input
data (per-core threshold scalars select ones/triangle/zero mask tiles).

Everything on the PE array is bf16 (fp32 HIGH poisons fast-weight-load
and fp32 moving operands stream at half rate). Score matmuls pack two
key-blocks per issue via PE row tiling: kT pairs live on partition
halves 0:64 / 64:128 (host interleaves ek into even/odd block regions),
qT is duplicated onto both halves by a column-duplicated Wq.

The program is a 4-phase pipeline, one phase per slot (processed small
to large: j=3,2,1,0). Phase p: DMA group for phase p+2, projections for
just the new kTp chunk / qT chunk / a quota of v blocks this slot needs,
4 mask tiles mk[t]=(QK>=th[t]) (QK is a gpsimd iota qi-kp-128*i2; th in
{-1e9,0,256,1e9} selects ones/triangle/zero), then the slot's attention.
Input DMAs are split across the SP and ACT issue queues; a handful of
dummy matmuls at the start keep the PE busy under the DMA wait so the
HAM clock-gate opens (1.2->2.4 GHz) before real work.

Per pair d (= template positions 2d, 2d+1) of slot j:
  ps[:,   0: 512] = kTp[ 0: 64, d].T @ qT[ 0: 64, slot]   (rows 0-63)
  ps[:, 512:1024] = kTp[64:128, d].T @ qT[64:128, slot]   (rows 64-127)
  e = exp(0.125*ps)                  bf16  (scalar engine)
  last 4 pairs: e *= mk[j,m]         (DVE tensor_mul)
  acc[:, slot] += e                  fp16  (DVE)
  po0 += v[u][:,0:128].T @ e_half;  po1 += v[u][:,128:256].T @ e_half
Outputs: outT bf16 [256,2048] (unnormalized), acc fp16 [128,4096].
Host: r = colsum(acc) folded over pair halves; out = (outT/r).T.
"""

import sys

sys.path.insert(0, "/opt/trn_rl_repo")

import numpy as np
import ml_dtypes

B, S, DM, DQ = 4, 4096, 256, 64
T = [32, 24, 16, 8]  # padded template: key-blocks per slot
STRIPES_A = [7, 5, 2, 0]  # R_A = [32, 24, 12, 4]
STRIPES_B = [6, 4, 3, 1]  # R_B = [28, 20, 16, 8]
# Per-pair mask thresholds for the last 4 pairs of a slot.
# exact slot (R == T): pairs are [ones, ones, tri(0/128), tri(256/384)]
# padded slot (R == T-4): pairs are [tri(0/128), tri(256/384), zero, zero]
TH_EXACT = [-1e9, -1e9, 0.0, 256.0]
TH_PAD = [0.0, 256.0, 1e9, 1e9]

_CACHE = {}


def _build_nc():
    import concourse.bass as bass  # noqa: F401
    import concourse.tile as tile
    from concourse import bacc, mybir

    dt = mybir.dt
    f32, bf, f16 = dt.float32, dt.bfloat16, dt.float16

    nc = bacc.Bacc(
        "TRN2",
        target_bir_lowering=False,
        debug=False,
        enable_asserts=False,
        num_devices=8,
    )

    def din(name, shape, d):
        return nc.dram_tensor(name, shape, d, kind="ExternalInput").ap()

    eq = din("eq", [256, 2048], bf)
    ek = din("ek", [256, 4096], bf)  # column-reordered: even blocks, then odd
    ev = din("ev", [256, 4096], bf)
    wq = din("wq", [256, 128], bf)  # Wq.T duplicated along cols
    wk = din("wk", [256, 64], bf)  # Wk.T
    wv = din("wv", [256, 256], bf)  # Wv.T
    th = din("th", [128, 16], f32)  # mask thresholds per (slot, pair)
    outT = nc.dram_tensor("outT", [256, 2048], bf, kind="ExternalOutput").ap()
    acc_out = nc.dram_tensor("acc", [128, 4096], f16, kind="ExternalOutput").ap()

    Exp = mybir.ActivationFunctionType.Exp
    GE = mybir.AluOpType.is_ge
    MUL = mybir.AluOpType.mult

    with tile.TileContext(nc) as tc:
        from contextlib import ExitStack

        with ExitStack() as ctx:
            const = ctx.enter_context(tc.tile_pool(name="const", bufs=1))

            # ---- persistent SBUF tensors ----
            eq_sb = [const.tile([128, 2048], bf, tag=f"eq{h}", name=f"eq{h}") for h in range(2)]
            ek_sb = [const.tile([128, 4096], bf, tag=f"ek{h}", name=f"ek{h}") for h in range(2)]
            ev_sb = [const.tile([128, 4096], bf, tag=f"ev{h}", name=f"ev{h}") for h in range(2)]
            wq_sb = const.tile([128, 256], bf, tag="wq", name="wq")
            wk_sb = const.tile([128, 128], bf, tag="wk", name="wk")
            wv_sb = const.tile([128, 512], bf, tag="wv", name="wv")
            th_sb = const.tile([128, 16], f32, tag="th", name="th")
            qT = const.tile([128, 2048], bf, tag="qT", name="qT")  # dup halves
            kTp = const.tile([128, 2048], bf, tag="kTp", name="kTp")  # pair-packed
            v_sb = const.tile([128, 32 * 256], bf, tag="v", name="v")
            qk = const.tile([128, 1024], f16, tag="qk", name="qk")
            acc = const.tile([128, 4096], f16, tag="acc", name="acc")
            mk = const.tile([128, 16 * 1024], bf, tag="mk", name="mk")

            # Input DMAs are issued from both SP and Activation queues (half
            # each) and staged per phase: phases 0-1 up front, later phases
            # from inside the pipeline so issue time hides under compute.
            def dma_phase(p, j):
                # Phase 0 splits issues across SP and ACT queues (shorter
                # critical path); later phases go all-SP so ACT stays free
                # for the exp stream.
                alt = nc.scalar if p == 0 else nc.sync
                cs_q = slice(j * 512, (j + 1) * 512)
                nc.sync.dma_start(eq_sb[0][:, cs_q], eq[0:128, cs_q])
                alt.dma_start(eq_sb[1][:, cs_q], eq[128:256, cs_q])
                if p == 0:
                    for h in range(2):
                        nc.sync.dma_start(
                            wq_sb[:, h * 128 : (h + 1) * 128], wq[h * 128 : (h + 1) * 128, :]
                        )
                        nc.sync.dma_start(
                            wk_sb[:, h * 64 : (h + 1) * 64], wk[h * 128 : (h + 1) * 128, :]
                        )
                        nc.scalar.dma_start(
                            wv_sb[:, h * 256 : (h + 1) * 256], wv[h * 128 : (h + 1) * 128, :]
                        )
                    nc.scalar.dma_start(th_sb[:], th[:])
                for reg in range(2):  # 0: even region, 1: odd region
                    cs = slice(reg * 2048 + p * 512, reg * 2048 + (p + 1) * 512)
                    nc.sync.dma_start(ek_sb[0][:, cs], ek[0:128, cs])
                    alt.dma_start(ek_sb[1][:, cs], ek[128:256, cs])
                for cc in EV_CH[p]:
                    cs = slice(cc * 512, (cc + 1) * 512)
                    nc.sync.dma_start(ev_sb[0][:, cs], ev[0:128, cs])
                    alt.dma_start(ev_sb[1][:, cs], ev[128:256, cs])

            PHASES = ((0, 3), (1, 2), (2, 1), (3, 0))
            V_QUOTA = ((0, 1, 2, 3, 4, 5), (6, 7, 8, 9), (10, 11, 12, 13), (14, 15))
            EV_CH = ((0, 1, 2), (3, 4), (5, 6), (7,))
            dma_phase(0, 3)
            dma_phase(1, 2)

            # Warm-up: dummy matmuls on scratch SBUF keep the PE busy while
            # input DMAs land, so the HAM clock-gate opens (1.2 -> 2.4 GHz)
            # before real work starts.
            scr = const.tile([128, 512], bf, tag="scr", name="scr")
            nc.gpsimd.memset(scr[:], 0.0)

            # QK[p, i2*512 + qi] = qi - 128*i2 - p
            nc.gpsimd.iota(
                qk[:],
                [[-128, 2], [1, 512]],
                base=0,
                channel_multiplier=-1,
                allow_small_or_imprecise_dtypes=True,
            )

            pp = ctx.enter_context(tc.tile_pool(name="pp", bufs=2, space="PSUM"))
            psc = ctx.enter_context(tc.tile_pool(name="psc", bufs=2, space="PSUM"))
            po_pool = ctx.enter_context(tc.tile_pool(name="po", bufs=1, space="PSUM"))
            epool = ctx.enter_context(tc.tile_pool(name="e", bufs=8))
            opool = ctx.enter_context(tc.tile_pool(name="o", bufs=2))

            for _ in range(6):
                ps = pp.tile([128, 512], f32, tag="ps", name="ps")
                nc.tensor.matmul(ps[:], scr[:, 0:128], scr[:], start=True, stop=True)

            LAG = 3
            # Phase p: projections for slot j=(3,2,1,0)[p], then attention on
            # slot j. Each phase's projections cover exactly the new kTp/v
            # columns that slot needs, so DMA/proj/attention pipeline.
            for p, j in PHASES:
                # qT chunk j (duplicated onto both halves by the dup'd wq)
                ps = pp.tile([128, 512], f32, tag="ps", name="ps")
                for h in range(2):
                    nc.tensor.matmul(
                        ps[:],
                        wq_sb[:, h * 128 : (h + 1) * 128],
                        eq_sb[h][:, j * 512 : (j + 1) * 512],
                        start=(h == 0),
                        stop=(h == 1),
                    )
                nc.vector.tensor_copy(qT[:, j * 512 : (j + 1) * 512], ps[:])
                # kTp chunk p: even blocks -> partitions 0:64, odd -> 64:128
                ps = pp.tile([128, 512], f32, tag="ps", name="ps")
                for half in range(2):
                    dst = ps[half * 64 : (half + 1) * 64, :]
                    for h in range(2):
                        nc.tensor.matmul(
                            dst,
                            wk_sb[:, h * 64 : (h + 1) * 64],
                            ek_sb[h][:, half * 2048 + p * 512 : half * 2048 + (p + 1) * 512],
                            start=(h == 0),
                            stop=(h == 1),
                        )
                nc.vector.tensor_copy(kTp[:, p * 512 : (p + 1) * 512], ps[:])
                # v pairs for this phase (natural [keys, 256] bf16); copies
                # alternate DVE/ACT so the pp psum pool recycles fast enough
                for n, i in enumerate(V_QUOTA[p]):
                    ps = pp.tile([128, 512], f32, tag="ps", name="ps")
                    for s in range(2):
                        t = 2 * i + s
                        for h in range(2):
                            nc.tensor.matmul(
                                ps[:, s * 256 : (s + 1) * 256],
                                ev_sb[h][:, t * 128 : (t + 1) * 128],
                                wv_sb[:, h * 256 : (h + 1) * 256],
                                start=(h == 0),
                                stop=(h == 1),
                            )
                    dst = v_sb[:, i * 512 : (i + 1) * 512]
                    if n % 2 == 0:
                        nc.scalar.copy(dst, ps[:])
                    else:
                        nc.vector.tensor_copy(dst, ps[:])
                # mask tiles for this slot: mk[t] = (QK >= th[t]) in bf16
                for m in range(4):
                    t = j * 4 + m
                    nc.vector.tensor_scalar(
                        mk[:, t * 1024 : (t + 1) * 1024],
                        qk[:],
                        th_sb[:, t : t + 1],
                        None,
                        GE,
                    )

                # ---- attention slot j ----
                npairs = T[j] // 2
                po0 = po_pool.tile([128, 512], f32, tag="po0", name="po0")
                po1 = po_pool.tile([128, 512], f32, tag="po1", name="po1")
                qs_top = qT[0:64, j * 512 : (j + 1) * 512]
                qs_bot = qT[64:128, j * 512 : (j + 1) * 512]
                acc_j = acc[:, j * 1024 : (j + 1) * 1024]
                es = [None] * npairs
                for d in range(npairs + LAG):
                    # pv first: its e input is LAG pairs old and guaranteed
                    # ready, so a scores matmul stalled on the psc buffer
                    # can't head-of-line-block it in the in-order PE queue.
                    if d >= LAG:
                        dd = d - LAG
                        e = es[dd]
                        for half in range(2):
                            u = 2 * dd + half
                            eh = e[:, half * 512 : (half + 1) * 512]
                            first = dd == 0 and half == 0
                            last = dd == npairs - 1 and half == 1
                            nc.tensor.matmul(
                                po0[:],
                                v_sb[:, u * 256 : u * 256 + 128],
                                eh,
                                start=first,
                                stop=last,
                            )
                            nc.tensor.matmul(
                                po1[:],
                                v_sb[:, u * 256 + 128 : (u + 1) * 256],
                                eh,
                                start=first,
                                stop=last,
                            )
                    if d < npairs:
                        ps = psc.tile([128, 1024], f32, tag="ps", name="ps")
                        nc.tensor.matmul(
                            ps[:, 0:512],
                            kTp[0:64, d * 128 : (d + 1) * 128],
                            qs_top,
                            start=True,
                            stop=True,
                        )
                        nc.tensor.matmul(
                            ps[:, 512:1024],
                            kTp[64:128, d * 128 : (d + 1) * 128],
                            qs_bot,
                            start=True,
                            stop=True,
                        )
                        e = epool.tile([128, 1024], bf, tag="e", name="e")
                        nc.scalar.activation(e[:], ps[:], Exp, scale=0.125)
                        if d >= npairs - 4:
                            m = d - (npairs - 4)
                            t = j * 4 + m
                            nc.vector.tensor_mul(
                                e[:], e[:], mk[:, t * 1024 : (t + 1) * 1024]
                            )
                        if d == 0:
                            nc.vector.tensor_copy(acc_j, e[:])
                        else:
                            nc.vector.tensor_add(acc_j, acc_j, e[:])
                        es[d] = e
                nc.sync.dma_start(acc_out[:, j * 1024 : (j + 1) * 1024], acc_j)
                o0 = opool.tile([128, 512], bf, tag="o0", name="o0")
                o1 = opool.tile([128, 512], bf, tag="o1", name="o1")
                nc.scalar.copy(o0[:], po0[:])
                if p == 3:  # final slot: parallelize the two output copies
                    nc.vector.tensor_copy(o1[:], po1[:])
                else:
                    nc.scalar.copy(o1[:], po1[:])
                nc.sync.dma_start(outT[0:128, j * 512 : (j + 1) * 512], o0[:])
                nc.sync.dma_start(outT[128:256, j * 512 : (j + 1) * 512], o1[:])
                if p + 2 <= 3:
                    dma_phase(*PHASES[p + 2])

    nc.compile()
    return nc


def kernel(encodings_for_q, encodings_for_k, encodings_for_v, mask, Wq, Wk, Wv):
    from concourse.bass_utils import run_bass_kernel_spmd

    if "nc" not in _CACHE:
        _CACHE["nc"] = _build_nc()
    nc = _CACHE["nc"]

    bf = ml_dtypes.bfloat16
    wqd = np.ascontiguousarray(
        np.concatenate([Wq.T, Wq.T], axis=1), dtype=bf
    )  # [256,128]
    wkt = np.ascontiguousarray(Wk.T, dtype=bf)  # [256,64]
    wvt = np.ascontiguousarray(Wv.T, dtype=bf)  # [256,256]

    in_maps = []
    metas = []
    for c in range(8):
        b, t = c // 2, c % 2
        stripes = STRIPES_A if t == 0 else STRIPES_B
        eqT = np.concatenate(
            [encodings_for_q[b, st * 512 : (st + 1) * 512, :].T for st in stripes],
            axis=1,
        )
        ekT = encodings_for_k[b].T.reshape(256, 32, 128)
        ek_reord = np.concatenate([ekT[:, 0::2, :], ekT[:, 1::2, :]], axis=1).reshape(
            256, 4096
        )
        # thresholds: slot j exact if R[j] == T[j]
        thr = np.empty((16,), dtype=np.float32)
        for j in range(4):
            R = 4 * (stripes[j] + 1)
            vals = TH_EXACT if R == T[j] else TH_PAD
            thr[j * 4 : (j + 1) * 4] = vals
        in_maps.append(
            {
                "eq": np.ascontiguousarray(eqT, dtype=bf),
                "ek": np.ascontiguousarray(ek_reord, dtype=bf),
                "ev": np.ascontiguousarray(encodings_for_v[b].T, dtype=bf),
                "wq": wqd,
                "wk": wkt,
                "wv": wvt,
                "th": np.ascontiguousarray(np.broadcast_to(thr, (128, 16))),
            }
        )
        metas.append((b, stripes))

    res = run_bass_kernel_spmd(nc, in_maps, core_ids=list(range(8)))
    _CACHE["last_res"] = res

    out = np.empty((B, S, DM), dtype=np.float32)
    for c in range(8):
        b, stripes = metas[c]
        oT = res.results[c]["outT"].astype(np.float32)
        a = res.results[c]["acc"].astype(np.float32)
        for j, st in enumerate(stripes):
            r = a[:, j * 1024 : j * 1024 + 512].sum(0) + a[
                :, j * 1024 + 512 : (j + 1) * 1024
            ].sum(0)
            blk = oT[:, j * 512 : (j + 1) * 512] / r[None, :]
            out[b, st * 512 : (st + 1) * 512, :] = blk.T
    return out


# revision 57
# speedup vs baseline: 1.1611x; 1.0018x over previous
"""Causal attention head on 8 trn2 NeuronCores.

Sharding: core c = (batch b = c//2, type t = c%2). Each core handles 4
query stripes of 512 of its batch. Causal balance: type A gets stripes
[7,5,2,0] with real key-block counts R_A=[32,24,12,4]; type B stripes
[6,4,3,1] with R_B=[28,20,16,8]. One SPMD program: every core runs the
padded template T=[32,24,16,8]; per-core behaviour comes only from input
data (per-core threshold scalars select ones/triangle/zero mask tiles).

Everything on the PE array is bf16 (fp32 HIGH poisons fast-weight-load
and fp32 moving operands stream at half rate). Score matmuls pack two
key-blocks per issue via PE row tiling: kT pairs live on partition
halves 0:64 / 64:128 (host interleaves ek into even/odd block regions),
qT is duplicated onto both halves by a column-duplicated Wq.

The program is a 4-phase pipeline, one phase per slot (processed small
to large: j=3,2,1,0). Phase p: DMA group for phase p+2, projections for
just the new kTp chunk / qT chunk / a quota of v blocks this slot needs,
4 mask tiles mk[t]=(QK>=th[t]) (QK is a gpsimd iota qi-kp-128*i2; th in
{-1e9,0,256,1e9} selects ones/triangle/zero), then the slot's attention.
Input DMAs are split across the SP and ACT issue queues; a handful of
dummy matmuls at the start keep the PE busy under the DMA wait so the
HAM clock-gate opens (1.2->2.4 GHz) before real work.

Per pair d (= template positions 2d, 2d+1) of slot j:
  ps[:,   0: 512] = kTp[ 0: 64, d].T @ qT[ 0: 64, slot]   (rows 0-63)
  ps[:, 512:1024] = kTp[64:128, d].T @ qT[64:128, slot]   (rows 64-127)
  e = exp(0.125*ps)                  bf16  (scalar engine)
  last 4 pairs: e *= mk[j,m]         (DVE tensor_mul)
  acc[:, slot] += e                  fp16  (DVE)
  po0 += v[u][:,0:128].T @ e_half;  po1 += v[u][:,128:256].T @ e_half
Outputs: outT bf16 [256,2048] (unnormalized), acc fp16 [128,4096].
Host: r = colsum(acc) folded over pair halves; out = (outT/r).T.
"""

import sys

sys.path.insert(0, "/opt/trn_rl_repo")

import numpy as np
import ml_dtypes

B, S, DM, DQ = 4, 4096, 256, 64
T = [32, 24, 16, 8]  # padded template: key-blocks per slot
STRIPES_A = [7, 5, 2, 0]  # R_A = [32, 24, 12, 4]
STRIPES_B = [6, 4, 3, 1]  # R_B = [28, 20, 16, 8]
# Per-pair mask thresholds for the last 4 pairs of a slot.
# exact slot (R == T): pairs are [ones, ones, tri(0/128), tri(256/384)]
# padded slot (R == T-4): pairs are [tri(0/128), tri(256/384), zero, zero]
TH_EXACT = [-1e9, -1e9, 0.0, 256.0]
TH_PAD = [0.0, 256.0, 1e9, 1e9]

_CACHE = {}


def _build_nc():
    import concourse.bass as bass  # noqa: F401
    import concourse.tile as tile
    from concourse import bacc, mybir

    dt = mybir.dt
    f32, bf, f16 = dt.float32, dt.bfloat16, dt.float16

    nc = bacc.Bacc(
        "TRN2",
        target_bir_lowering=False,
        debug=False,
        enable_asserts=False,
        num_devices=8,
    )

    def din(name, shape, d):
        return nc.dram_tensor(name, shape, d, kind="ExternalInput").ap()

    eq = din("eq", [256, 2048], bf)
    ek = din("ek", [256, 4096], bf)  # column-reordered: even blocks, then odd
    ev = din("ev", [256, 4096], bf)
    wq = din("wq", [256, 128], bf)  # Wq.T duplicated along cols
    wk = din("wk", [256, 64], bf)  # Wk.T
    wv = din("wv", [256, 256], bf)  # Wv.T
    th = din("th", [128, 16], f32)  # mask thresholds per (slot, pair)
    outT = nc.dram_tensor("outT", [256, 2048], bf, kind="ExternalOutput").ap()
    acc_out = nc.dram_tensor("acc", [128, 4096], f16, kind="ExternalOutput").ap()

    Exp = mybir.ActivationFunctionType.Exp
    GE = mybir.AluOpType.is_ge
    MUL = mybir.AluOpType.mult

    with tile.TileContext(nc) as tc:
        from contextlib import ExitStack

        with ExitStack() as ctx:
            const = ctx.enter_context(tc.tile_pool(name="const", bufs=1))

            # ---- persistent SBUF tensors ----
            eq_sb = [const.tile([128, 2048], bf, tag=f"eq{h}", name=f"eq{h}") for h in range(2)]
            ek_sb = [const.tile([128, 4096], bf, tag=f"ek{h}", name=f"ek{h}") for h in range(2)]
            ev_sb = [const.tile([128, 4096], bf, tag=f"ev{h}", name=f"ev{h}") for h in range(2)]
            wq_sb = const.tile([128, 256], bf, tag="wq", name="wq")
            wk_sb = const.tile([128, 128], bf, tag="wk", name="wk")
            wv_sb = const.tile([128, 512], bf, tag="wv", name="wv")
            th_sb = const.tile([128, 16], f32, tag="th", name="th")
            qT = const.tile([128, 2048], bf, tag="qT", name="qT")  # dup halves
            kTp = const.tile([128, 2048], bf, tag="kTp", name="kTp")  # pair-packed
            v_sb = const.tile([128, 32 * 256], bf, tag="v", name="v")
            qk = const.tile([128, 1024], f16, tag="qk", name="qk")
            acc = const.tile([128, 4096], f16, tag="acc", name="acc")
            mk = const.tile([128, 16 * 1024], bf, tag="mk", name="mk")

            # Input DMAs are issued from both SP and Activation queues (half
            # each) and staged per phase: phases 0-1 up front, later phases
            # from inside the pipeline so issue time hides under compute.
            def dma_phase(p, j):
                # Phase 0 splits issues across SP and ACT queues (shorter
                # critical path); later phases go all-SP so ACT stays free
                # for the exp stream.
                alt = nc.scalar if p == 0 else nc.sync
                cs_q = slice(j * 512, (j + 1) * 512)
                nc.sync.dma_start(eq_sb[0][:, cs_q], eq[0:128, cs_q])
                alt.dma_start(eq_sb[1][:, cs_q], eq[128:256, cs_q])
                if p == 0:
                    for h in range(2):
                        nc.sync.dma_start(
                            wq_sb[:, h * 128 : (h + 1) * 128], wq[h * 128 : (h + 1) * 128, :]
                        )
                        nc.sync.dma_start(
                            wk_sb[:, h * 64 : (h + 1) * 64], wk[h * 128 : (h + 1) * 128, :]
                        )
                        nc.scalar.dma_start(
                            wv_sb[:, h * 256 : (h + 1) * 256], wv[h * 128 : (h + 1) * 128, :]
                        )
                    nc.scalar.dma_start(th_sb[:], th[:])
                for reg in range(2):  # 0: even region, 1: odd region
                    cs = slice(reg * 2048 + p * 512, reg * 2048 + (p + 1) * 512)
                    nc.sync.dma_start(ek_sb[0][:, cs], ek[0:128, cs])
                    alt.dma_start(ek_sb[1][:, cs], ek[128:256, cs])
                for cc in EV_CH[p]:
                    cs = slice(cc * 512, (cc + 1) * 512)
                    nc.sync.dma_start(ev_sb[0][:, cs], ev[0:128, cs])
                    alt.dma_start(ev_sb[1][:, cs], ev[128:256, cs])

            PHASES = ((0, 3), (1, 2), (2, 1), (3, 0))
            V_QUOTA = ((0, 1, 2, 3, 4, 5), (6, 7, 8, 9), (10, 11, 12, 13), (14, 15))
            EV_CH = ((0, 1, 2), (3, 4), (5, 6), (7,))
            dma_phase(0, 3)
            dma_phase(1, 2)

            # Warm-up: dummy matmuls on scratch SBUF keep the PE busy while
            # input DMAs land, so the HAM clock-gate opens (1.2 -> 2.4 GHz)
            # before real work starts.
            scr = const.tile([128, 512], bf, tag="scr", name="scr")
            nc.gpsimd.memset(scr[:], 0.0)

            # QK[p, i2*512 + qi] = qi - 128*i2 - p
            nc.gpsimd.iota(
                qk[:],
                [[-128, 2], [1, 512]],
                base=0,
                channel_multiplier=-1,
                allow_small_or_imprecise_dtypes=True,
            )

            pp = ctx.enter_context(tc.tile_pool(name="pp", bufs=2, space="PSUM"))
            psc = ctx.enter_context(tc.tile_pool(name="psc", bufs=2, space="PSUM"))
            po_pool = ctx.enter_context(tc.tile_pool(name="po", bufs=1, space="PSUM"))
            epool = ctx.enter_context(tc.tile_pool(name="e", bufs=8))
            opool = ctx.enter_context(tc.tile_pool(name="o", bufs=2))

            for _ in range(6):
                ps = pp.tile([128, 512], f32, tag="ps", name="ps")
                nc.tensor.matmul(ps[:], scr[:, 0:128], scr[:], start=True, stop=True)

            LAG = 3
            # Phase p: projections for slot j=(3,2,1,0)[p], then attention on
            # slot j. Each phase's projections cover exactly the new kTp/v
            # columns that slot needs, so DMA/proj/attention pipeline.
            for p, j in PHASES:
                # qT chunk j (duplicated onto both halves by the dup'd wq)
                ps = pp.tile([128, 512], f32, tag="ps", name="ps")
                for h in range(2):
                    nc.tensor.matmul(
                        ps[:],
                        wq_sb[:, h * 128 : (h + 1) * 128],
                        eq_sb[h][:, j * 512 : (j + 1) * 512],
                        start=(h == 0),
                        stop=(h == 1),
                    )
                nc.vector.tensor_copy(qT[:, j * 512 : (j + 1) * 512], ps[:])
                # kTp chunk p: even blocks -> partitions 0:64, odd -> 64:128
                ps = pp.tile([128, 512], f32, tag="ps", name="ps")
                for half in range(2):
                    dst = ps[half * 64 : (half + 1) * 64, :]
                    for h in range(2):
                        nc.tensor.matmul(
                            dst,
                            wk_sb[:, h * 64 : (h + 1) * 64],
                            ek_sb[h][:, half * 2048 + p * 512 : half * 2048 + (p + 1) * 512],
                            start=(h == 0),
                            stop=(h == 1),
                        )
                nc.vector.tensor_copy(kTp[:, p * 512 : (p + 1) * 512], ps[:])
                # v pairs for this phase (natural [keys, 256] bf16); copies
                # alternate DVE/ACT so the pp psum pool recycles fast enough
                for n, i in enumerate(V_QUOTA[p]):
                    ps = pp.tile([128, 512], f32, tag="ps", name="ps")
                    for s in range(2):
                        t = 2 * i + s
                        for h in range(2):
                            nc.tensor.matmul(
                                ps[:, s * 256 : (s + 1) * 256],
                                ev_sb[h][:, t * 128 : (t + 1) * 128],
                                wv_sb[:, h * 256 : (h + 1) * 256],
                                start=(h == 0),
                                stop=(h == 1),
                            )
                    dst = v_sb[:, i * 512 : (i + 1) * 512]
                    if n % 2 == 0:
                        nc.scalar.copy(dst, ps[:])
                    else:
                        nc.vector.tensor_copy(dst, ps[:])
                # mask tiles for this slot: mk[t] = (QK >= th[t]) in bf16
                for m in range(4):
                    t = j * 4 + m
                    nc.vector.tensor_scalar(
                        mk[:, t * 1024 : (t + 1) * 1024],
                        qk[:],
                        th_sb[:, t : t + 1],
                        None,
                        GE,
                    )

                # ---- attention slot j ----
                npairs = T[j] // 2
                po0 = po_pool.tile([128, 512], f32, tag="po0", name="po0")
                po1 = po_pool.tile([128, 512], f32, tag="po1", name="po1")
                qs_top = qT[0:64, j * 512 : (j + 1) * 512]
                qs_bot = qT[64:128, j * 512 : (j + 1) * 512]
                acc_j = acc[:, j * 1024 : (j + 1) * 1024]
                es = [None] * npairs
                # Final slot: process masked pairs mid-slot so the very last
                # pv chain has no DVE mask dependency in the drain.
                if p == 3:
                    proc = list(range(8)) + [12, 13, 14, 15] + [8, 9, 10, 11]
                else:
                    proc = list(range(npairs))
                for d in range(npairs + LAG):
                    # pv first: its e input is LAG pairs old and guaranteed
                    # ready, so a scores matmul stalled on the psc buffer
                    # can't head-of-line-block it in the in-order PE queue.
                    if d >= LAG:
                        dd = d - LAG
                        e = es[dd]
                        for half in range(2):
                            u = 2 * proc[dd] + half
                            eh = e[:, half * 512 : (half + 1) * 512]
                            first = dd == 0 and half == 0
                            last = dd == npairs - 1 and half == 1
                            nc.tensor.matmul(
                                po0[:],
                                v_sb[:, u * 256 : u * 256 + 128],
                                eh,
                                start=first,
                                stop=last,
                            )
                            nc.tensor.matmul(
                                po1[:],
                                v_sb[:, u * 256 + 128 : (u + 1) * 256],
                                eh,
                                start=first,
                                stop=last,
                            )
                    if d < npairs:
                        tp = proc[d]
                        ps = psc.tile([128, 1024], f32, tag="ps", name="ps")
                        nc.tensor.matmul(
                            ps[:, 0:512],
                            kTp[0:64, tp * 128 : (tp + 1) * 128],
                            qs_top,
                            start=True,
                            stop=True,
                        )
                        nc.tensor.matmul(
                            ps[:, 512:1024],
                            kTp[64:128, tp * 128 : (tp + 1) * 128],
                            qs_bot,
                            start=True,
                            stop=True,
                        )
                        e = epool.tile([128, 1024], bf, tag="e", name="e")
                        nc.scalar.activation(e[:], ps[:], Exp, scale=0.125)
                        if tp >= npairs - 4:
                            m = tp - (npairs - 4)
                            t = j * 4 + m
                            nc.vector.tensor_mul(
                                e[:], e[:], mk[:, t * 1024 : (t + 1) * 1024]
                            )
                        if d == 0:
                            nc.vector.tensor_copy(acc_j, e[:])
                        else:
                            nc.vector.tensor_add(acc_j, acc_j, e[:])
                        es[d] = e
                nc.sync.dma_start(acc_out[:, j * 1024 : (j + 1) * 1024], acc_j)
                o0 = opool.tile([128, 512], bf, tag="o0", name="o0")
                o1 = opool.tile([128, 512], bf, tag="o1", name="o1")
                nc.scalar.copy(o0[:], po0[:])
                if p == 3:  # final slot: parallelize the two output copies
                    nc.vector.tensor_copy(o1[:], po1[:])
                else:
                    nc.scalar.copy(o1[:], po1[:])
                nc.sync.dma_start(outT[0:128, j * 512 : (j + 1) * 512], o0[:])
                if p == 3:  # final slot: second out DMA from the ACT queue
                    nc.scalar.dma_start(outT[128:256, j * 512 : (j + 1) * 512], o1[:])
                else:
                    nc.sync.dma_start(outT[128:256, j * 512 : (j + 1) * 512], o1[:])
                if p + 2 <= 3:
                    dma_phase(*PHASES[p + 2])

    nc.compile()
    return nc


def kernel(encodings_for_q, encodings_for_k, encodings_for_v, mask, Wq, Wk, Wv):
    from concourse.bass_utils import run_bass_kernel_spmd

    if "nc" not in _CACHE:
        _CACHE["nc"] = _build_nc()
    nc = _CACHE["nc"]

    bf = ml_dtypes.bfloat16
    wqd = np.ascontiguousarray(
        np.concatenate([Wq.T, Wq.T], axis=1), dtype=bf
    )  # [256,128]
    wkt = np.ascontiguousarray(Wk.T, dtype=bf)  # [256,64]
    wvt = np.ascontiguousarray(Wv.T, dtype=bf)  # [256,256]

    in_maps = []
    metas = []
    for c in range(8):
        b, t = c // 2, c % 2
        stripes = STRIPES_A if t == 0 else STRIPES_B
        eqT = np.concatenate(
            [encodings_for_q[b, st * 512 : (st + 1) * 512, :].T for st in stripes],
            axis=1,
        )
        ekT = encodings_for_k[b].T.reshape(256, 32, 128)
        ek_reord = np.concatenate([ekT[:, 0::2, :], ekT[:, 1::2, :]], axis=1).reshape(
            256, 4096
        )
        # thresholds: slot j exact if R[j] == T[j]
        thr = np.empty((16,), dtype=np.float32)
        for j in range(4):
            R = 4 * (stripes[j] + 1)
            vals = TH_EXACT if R == T[j] else TH_PAD
            thr[j * 4 : (j + 1) * 4] = vals
        in_maps.append(
            {
                "eq": np.ascontiguousarray(eqT, dtype=bf),
                "ek": np.ascontiguousarray(ek_reord, dtype=bf),
                "ev": np.ascontiguousarray(encodings_for_v[b].T, dtype=bf),
                "wq": wqd,
                "wk": wkt,
                "wv": wvt,
                "th": np.ascontiguousarray(np.broadcast_to(thr, (128, 16))),
            }
        )
        metas.append((b, stripes))

    res = run_bass_kernel_spmd(nc, in_maps, core_ids=list(range(8)))
    _CACHE["last_res"] = res

    out = np.empty((B, S, DM), dtype=np.float32)
    for c in range(8):
        b, stripes = metas[c]
        oT = res.results[c]["outT"].astype(np.float32)
        a = res.results[c]["acc"].astype(np.float32)
        for j, st in enumerate(stripes):
            r = a[:, j * 1024 : j * 1024 + 512].sum(0) + a[
                :, j * 1024 + 512 : (j + 1) * 1024
            ].sum(0)
            blk = oT[:, j * 512 : (j + 1) * 512] / r[None, :]
            out[b, st * 512 : (st + 1) * 512, :] = blk.T
    return out


# revision 59
# speedup vs baseline: 1.1625x; 1.0012x over previous
"""Causal attention head on 8 trn2 NeuronCores.

Sharding: core c = (batch b = c//2, type t = c%2). Each core handles 4
query stripes of 512 of its batch. Causal balance: type A gets stripes
[7,5,2,0] with real key-block counts R_A=[32,24,12,4]; type B stripes
[6,4,3,1] with R_B=[28,20,16,8]. One SPMD program: every core runs the
padded template T=[32,24,16,8]; per-core behaviour comes only from input
data (per-core threshold scalars select ones/triangle/zero mask tiles).

Everything on the PE array is bf16 (fp32 HIGH poisons fast-weight-load
and fp32 moving operands stream at half rate). Score matmuls pack two
key-blocks per issue via PE row tiling: kT pairs live on partition
halves 0:64 / 64:128 (host interleaves ek into even/odd block regions),
qT is duplicated onto both halves by a column-duplicated Wq.

The program is a 4-phase pipeline, one phase per slot (processed small
to large: j=3,2,1,0). Phase p: DMA group for phase p+2, projections for
just the new kTp chunk / qT chunk / a quota of v blocks this slot needs,
4 mask tiles mk[t]=(QK>=th[t]) (QK is a gpsimd iota qi-kp-128*i2; th in
{-1e9,0,256,1e9} selects ones/triangle/zero), then the slot's attention.
Input DMAs are split across the SP and ACT issue queues; a handful of
dummy matmuls at the start keep the PE busy under the DMA wait so the
HAM clock-gate opens (1.2->2.4 GHz) before real work.

Per pair d (= template positions 2d, 2d+1) of slot j:
  ps[:,   0: 512] = kTp[ 0: 64, d].T @ qT[ 0: 64, slot]   (rows 0-63)
  ps[:, 512:1024] = kTp[64:128, d].T @ qT[64:128, slot]   (rows 64-127)
  e = exp(0.125*ps)                  bf16  (scalar engine)
  last 4 pairs: e *= mk[j,m]         (DVE tensor_mul)
  acc[:, slot] += e                  fp16  (DVE)
  po0 += v[u][:,0:128].T @ e_half;  po1 += v[u][:,128:256].T @ e_half
Outputs: outT bf16 [256,2048] (unnormalized), acc fp16 [128,4096].
Host: r = colsum(acc) folded over pair halves; out = (outT/r).T.
"""

import sys

sys.path.insert(0, "/opt/trn_rl_repo")

import numpy as np
import ml_dtypes

B, S, DM, DQ = 4, 4096, 256, 64
T = [32, 24, 16, 8]  # padded template: key-blocks per slot
STRIPES_A = [7, 5, 2, 0]  # R_A = [32, 24, 12, 4]
STRIPES_B = [6, 4, 3, 1]  # R_B = [28, 20, 16, 8]
# Per-pair mask thresholds for the last 4 pairs of a slot.
# exact slot (R == T): pairs are [ones, ones, tri(0/128), tri(256/384)]
# padded slot (R == T-4): pairs are [tri(0/128), tri(256/384), zero, zero]
TH_EXACT = [-1e9, -1e9, 0.0, 256.0]
TH_PAD = [0.0, 256.0, 1e9, 1e9]

_CACHE = {}


def _build_nc():
    import concourse.bass as bass  # noqa: F401
    import concourse.tile as tile
    from concourse import bacc, mybir

    dt = mybir.dt
    f32, bf, f16 = dt.float32, dt.bfloat16, dt.float16

    nc = bacc.Bacc(
        "TRN2",
        target_bir_lowering=False,
        debug=False,
        enable_asserts=False,
        num_devices=8,
    )

    def din(name, shape, d):
        return nc.dram_tensor(name, shape, d, kind="ExternalInput").ap()

    eq = din("eq", [256, 2048], bf)
    ek = din("ek", [256, 4096], bf)  # column-reordered: even blocks, then odd
    ev = din("ev", [256, 4096], bf)
    wq = din("wq", [256, 128], bf)  # Wq.T duplicated along cols
    wk = din("wk", [256, 64], bf)  # Wk.T
    wv = din("wv", [256, 256], bf)  # Wv.T
    th = din("th", [128, 16], f32)  # mask thresholds per (slot, pair)
    outT = nc.dram_tensor("outT", [256, 2048], bf, kind="ExternalOutput").ap()
    acc_out = nc.dram_tensor("acc", [128, 4096], f16, kind="ExternalOutput").ap()

    Exp = mybir.ActivationFunctionType.Exp
    GE = mybir.AluOpType.is_ge
    MUL = mybir.AluOpType.mult

    with tile.TileContext(nc) as tc:
        from contextlib import ExitStack

        with ExitStack() as ctx:
            const = ctx.enter_context(tc.tile_pool(name="const", bufs=1))

            # ---- persistent SBUF tensors ----
            eq_sb = [const.tile([128, 2048], bf, tag=f"eq{h}", name=f"eq{h}") for h in range(2)]
            ek_sb = [const.tile([128, 4096], bf, tag=f"ek{h}", name=f"ek{h}") for h in range(2)]
            ev_sb = [const.tile([128, 4096], bf, tag=f"ev{h}", name=f"ev{h}") for h in range(2)]
            wq_sb = const.tile([128, 256], bf, tag="wq", name="wq")
            wk_sb = const.tile([128, 128], bf, tag="wk", name="wk")
            wv_sb = const.tile([128, 512], bf, tag="wv", name="wv")
            th_sb = const.tile([128, 16], f32, tag="th", name="th")
            qT = const.tile([128, 2048], bf, tag="qT", name="qT")  # dup halves
            kTp = const.tile([128, 2048], bf, tag="kTp", name="kTp")  # pair-packed
            v_sb = const.tile([128, 32 * 256], bf, tag="v", name="v")
            qk = const.tile([128, 1024], f16, tag="qk", name="qk")
            acc = const.tile([128, 4096], f16, tag="acc", name="acc")
            mk = const.tile([128, 16 * 1024], bf, tag="mk", name="mk")

            # Input DMAs are issued from both SP and Activation queues (half
            # each) and staged per phase: phases 0-1 up front, later phases
            # from inside the pipeline so issue time hides under compute.
            def dma_phase(p, j):
                # Phase 0 splits issues across SP and ACT queues (shorter
                # critical path); later phases go all-SP so ACT stays free
                # for the exp stream.
                alt = nc.scalar if p == 0 else nc.sync
                cs_q = slice(j * 512, (j + 1) * 512)
                nc.sync.dma_start(eq_sb[0][:, cs_q], eq[0:128, cs_q])
                alt.dma_start(eq_sb[1][:, cs_q], eq[128:256, cs_q])
                if p == 0:
                    for h in range(2):
                        nc.sync.dma_start(
                            wq_sb[:, h * 128 : (h + 1) * 128], wq[h * 128 : (h + 1) * 128, :]
                        )
                        nc.sync.dma_start(
                            wk_sb[:, h * 64 : (h + 1) * 64], wk[h * 128 : (h + 1) * 128, :]
                        )
                        nc.scalar.dma_start(
                            wv_sb[:, h * 256 : (h + 1) * 256], wv[h * 128 : (h + 1) * 128, :]
                        )
                    nc.scalar.dma_start(th_sb[:], th[:])
                for reg in range(2):  # 0: even region, 1: odd region
                    cs = slice(reg * 2048 + p * 512, reg * 2048 + (p + 1) * 512)
                    nc.sync.dma_start(ek_sb[0][:, cs], ek[0:128, cs])
                    alt.dma_start(ek_sb[1][:, cs], ek[128:256, cs])
                for cc in EV_CH[p]:
                    cs = slice(cc * 512, (cc + 1) * 512)
                    nc.sync.dma_start(ev_sb[0][:, cs], ev[0:128, cs])
                    alt.dma_start(ev_sb[1][:, cs], ev[128:256, cs])

            PHASES = ((0, 3), (1, 2), (2, 1), (3, 0))
            V_QUOTA = ((0, 1, 2, 3, 4, 5), (6, 7, 8, 9), (10, 11, 12, 13), (14, 15))
            EV_CH = ((0, 1, 2), (3, 4), (5, 6), (7,))
            dma_phase(0, 3)
            dma_phase(1, 2)

            # Warm-up: dummy matmuls on scratch SBUF keep the PE busy while
            # input DMAs land, so the HAM clock-gate opens (1.2 -> 2.4 GHz)
            # before real work starts.
            scr = const.tile([128, 512], bf, tag="scr", name="scr")
            nc.gpsimd.memset(scr[:], 0.0)

            # QK[p, i2*512 + qi] = qi - 128*i2 - p
            nc.gpsimd.iota(
                qk[:],
                [[-128, 2], [1, 512]],
                base=0,
                channel_multiplier=-1,
                allow_small_or_imprecise_dtypes=True,
            )

            pp = ctx.enter_context(tc.tile_pool(name="pp", bufs=2, space="PSUM"))
            psc = ctx.enter_context(tc.tile_pool(name="psc", bufs=2, space="PSUM"))
            po_pool = ctx.enter_context(tc.tile_pool(name="po", bufs=1, space="PSUM"))
            epool = ctx.enter_context(tc.tile_pool(name="e", bufs=8))
            opool = ctx.enter_context(tc.tile_pool(name="o", bufs=2))

            for _ in range(6):
                ps = pp.tile([128, 512], f32, tag="ps", name="ps")
                nc.tensor.matmul(ps[:], scr[:, 0:128], scr[:], start=True, stop=True)

            LAG = 3
            # Phase p: projections for slot j=(3,2,1,0)[p], then attention on
            # slot j. Each phase's projections cover exactly the new kTp/v
            # columns that slot needs, so DMA/proj/attention pipeline.
            for p, j in PHASES:
                # qT chunk j (duplicated onto both halves by the dup'd wq)
                ps = pp.tile([128, 512], f32, tag="ps", name="ps")
                for h in range(2):
                    nc.tensor.matmul(
                        ps[:],
                        wq_sb[:, h * 128 : (h + 1) * 128],
                        eq_sb[h][:, j * 512 : (j + 1) * 512],
                        start=(h == 0),
                        stop=(h == 1),
                    )
                nc.vector.tensor_copy(qT[:, j * 512 : (j + 1) * 512], ps[:])
                # kTp chunk p: even blocks -> partitions 0:64, odd -> 64:128
                ps = pp.tile([128, 512], f32, tag="ps", name="ps")
                for half in range(2):
                    dst = ps[half * 64 : (half + 1) * 64, :]
                    for h in range(2):
                        nc.tensor.matmul(
                            dst,
                            wk_sb[:, h * 64 : (h + 1) * 64],
                            ek_sb[h][:, half * 2048 + p * 512 : half * 2048 + (p + 1) * 512],
                            start=(h == 0),
                            stop=(h == 1),
                        )
                nc.vector.tensor_copy(kTp[:, p * 512 : (p + 1) * 512], ps[:])
                # v pairs for this phase (natural [keys, 256] bf16); copies
                # alternate DVE/ACT so the pp psum pool recycles fast enough
                for n, i in enumerate(V_QUOTA[p]):
                    ps = pp.tile([128, 512], f32, tag="ps", name="ps")
                    for s in range(2):
                        t = 2 * i + s
                        for h in range(2):
                            nc.tensor.matmul(
                                ps[:, s * 256 : (s + 1) * 256],
                                ev_sb[h][:, t * 128 : (t + 1) * 128],
                                wv_sb[:, h * 256 : (h + 1) * 256],
                                start=(h == 0),
                                stop=(h == 1),
                            )
                    dst = v_sb[:, i * 512 : (i + 1) * 512]
                    if n % 2 == 0:
                        nc.scalar.copy(dst, ps[:])
                    else:
                        nc.vector.tensor_copy(dst, ps[:])
                # mask tiles for this slot: mk[t] = (QK >= th[t]) in bf16
                for m in range(4):
                    t = j * 4 + m
                    nc.vector.tensor_scalar(
                        mk[:, t * 1024 : (t + 1) * 1024],
                        qk[:],
                        th_sb[:, t : t + 1],
                        None,
                        GE,
                    )

                # ---- attention slot j ----
                npairs = T[j] // 2
                po0 = po_pool.tile([128, 512], f32, tag="po0", name="po0")
                po1 = po_pool.tile([128, 512], f32, tag="po1", name="po1")
                qs_top = qT[0:64, j * 512 : (j + 1) * 512]
                qs_bot = qT[64:128, j * 512 : (j + 1) * 512]
                acc_j = acc[:, j * 1024 : (j + 1) * 1024]
                es = [None] * npairs
                # Process masked pairs right after the pipeline fills (their
                # DVE mask-muls get full lag slack) and end every slot on
                # unmasked pairs so the drain chain has no DVE dependency.
                if npairs > 4:
                    k = min(4, npairs - 8)
                    proc = (
                        list(range(k))
                        + list(range(npairs - 4, npairs))
                        + list(range(k, npairs - 4))
                    )
                else:
                    proc = list(range(npairs))
                for d in range(npairs + LAG):
                    # pv first: its e input is LAG pairs old and guaranteed
                    # ready, so a scores matmul stalled on the psc buffer
                    # can't head-of-line-block it in the in-order PE queue.
                    if d >= LAG:
                        dd = d - LAG
                        e = es[dd]
                        for half in range(2):
                            u = 2 * proc[dd] + half
                            eh = e[:, half * 512 : (half + 1) * 512]
                            first = dd == 0 and half == 0
                            last = dd == npairs - 1 and half == 1
                            nc.tensor.matmul(
                                po0[:],
                                v_sb[:, u * 256 : u * 256 + 128],
                                eh,
                                start=first,
                                stop=last,
                            )
                            nc.tensor.matmul(
                                po1[:],
                                v_sb[:, u * 256 + 128 : (u + 1) * 256],
                                eh,
                                start=first,
                                stop=last,
                            )
                    if d < npairs:
                        tp = proc[d]
                        ps = psc.tile([128, 1024], f32, tag="ps", name="ps")
                        nc.tensor.matmul(
                            ps[:, 0:512],
                            kTp[0:64, tp * 128 : (tp + 1) * 128],
                            qs_top,
                            start=True,
                            stop=True,
                        )
                        nc.tensor.matmul(
                            ps[:, 512:1024],
                            kTp[64:128, tp * 128 : (tp + 1) * 128],
                            qs_bot,
                            start=True,
                            stop=True,
                        )
                        e = epool.tile([128, 1024], bf, tag="e", name="e")
                        nc.scalar.activation(e[:], ps[:], Exp, scale=0.125)
                        if tp >= npairs - 4:
                            m = tp - (npairs - 4)
                            t = j * 4 + m
                            nc.vector.tensor_mul(
                                e[:], e[:], mk[:, t * 1024 : (t + 1) * 1024]
                            )
                        if d == 0:
                            nc.vector.tensor_copy(acc_j, e[:])
                        else:
                            nc.vector.tensor_add(acc_j, acc_j, e[:])
                        es[d] = e
                nc.sync.dma_start(acc_out[:, j * 1024 : (j + 1) * 1024], acc_j)
                o0 = opool.tile([128, 512], bf, tag="o0", name="o0")
                o1 = opool.tile([128, 512], bf, tag="o1", name="o1")
                nc.scalar.copy(o0[:], po0[:])
                if p == 3:  # final slot: parallelize the two output copies
                    nc.vector.tensor_copy(o1[:], po1[:])
                else:
                    nc.scalar.copy(o1[:], po1[:])
                nc.sync.dma_start(outT[0:128, j * 512 : (j + 1) * 512], o0[:])
                if p == 3:  # final slot: second out DMA from the ACT queue
                    nc.scalar.dma_start(outT[128:256, j * 512 : (j + 1) * 512], o1[:])
                else:
                    nc.sync.dma_start(outT[128:256, j * 512 : (j + 1) * 512], o1[:])
                if p + 2 <= 3:
                    dma_phase(*PHASES[p + 2])

    nc.compile()
    return nc


def kernel(encodings_for_q, encodings_for_k, encodings_for_v, mask, Wq, Wk, Wv):
    from concourse.bass_utils import run_bass_kernel_spmd

    if "nc" not in _CACHE:
        _CACHE["nc"] = _build_nc()
    nc = _CACHE["nc"]

    bf = ml_dtypes.bfloat16
    wqd = np.ascontiguousarray(
        np.concatenate([Wq.T, Wq.T], axis=1), dtype=bf
    )  # [256,128]
    wkt = np.ascontiguousarray(Wk.T, dtype=bf)  # [256,64]
    wvt = np.ascontiguousarray(Wv.T, dtype=bf)  # [256,256]

    in_maps = []
    metas = []
    for c in range(8):
        b, t = c // 2, c % 2
        stripes = STRIPES_A if t == 0 else STRIPES_B
        eqT = np.concatenate(
            [encodings_for_q[b, st * 512 : (st + 1) * 512, :].T for st in stripes],
            axis=1,
        )
        ekT = encodings_for_k[b].T.reshape(256, 32, 128)
        ek_reord = np.concatenate([ekT[:, 0::2, :], ekT[:, 1::2, :]], axis=1).reshape(
            256, 4096
        )
        # thresholds: slot j exact if R[j] == T[j]
        thr = np.empty((16,), dtype=np.float32)
        for j in range(4):
            R = 4 * (stripes[j] + 1)
            vals = TH_EXACT if R == T[j] else TH_PAD
            thr[j * 4 : (j + 1) * 4] = vals
        in_maps.append(
            {
                "eq": np.ascontiguousarray(eqT, dtype=bf),
                "ek": np.ascontiguousarray(ek_reord, dtype=bf),
                "ev": np.ascontiguousarray(encodings_for_v[b].T, dtype=bf),
                "wq": wqd,
                "wk": wkt,
                "wv": wvt,
                "th": np.ascontiguousarray(np.broadcast_to(thr, (128, 16))),
            }
        )
        metas.append((b, stripes))

    res = run_bass_kernel_spmd(nc, in_maps, core_ids=list(range(8)))
    _CACHE["last_res"] = res

    out = np.empty((B, S, DM), dtype=np.float32)
    for c in range(8):
        b, stripes = metas[c]
        oT = res.results[c]["outT"].astype(np.float32)
        a = res.results[c]["acc"].astype(np.float32)
        for j, st in enumerate(stripes):
            r = a[:, j * 1024 : j * 1024 + 512].sum(0) + a[
                :, j * 1024 + 512 : (j + 1) * 1024
            ].sum(0)
            blk = oT[:, j * 512 : (j + 1) * 512] / r[None, :]
            out[b, st * 512 : (st + 1) * 512, :] = blk.T
    return out
